# revision 14
# baseline (speedup 1.0000x reference)
"""Multi-head attention block (B=2, N=2048, C=1024, H=16, hd=64) on 8 TRN2 NeuronCores.

Sharding: data-parallel over batch (2 groups of 4 cores), tensor-parallel over
heads within each group (4 heads/core). Each core computes q/k/v for its heads,
attention, and a partial output projection; a ReduceScatter over the 4-core
group sums the partials, and the host reassembles the full [2, 2048, 1024]
output from the per-core shards.

Per-core layouts (everything transposed so the contraction dim sits on SBUF
partitions; the host pre-transposes x):
  xt   [1024, 2048]  x[b].T
  wqk  [1024, 512]   w_qkv columns for this core's q (256) ++ k (256)
  wv   [1024, 256]   w_qkv columns for this core's v
  wpb  [256, 1024]   w_proj rows for this core's heads
  bq   [128, 8]      b_proj/4, bq[p, m] = b_proj[m*128+p]/4
  out  [256, 2048]   rows g*256:(g+1)*256 of (x[b] @ ... ).T after RS
"""
import sys

if '/opt/trn_rl_repo' not in sys.path:
    sys.path.insert(0, '/opt/trn_rl_repo')

import numpy as np

import concourse.bass as bass
import concourse.mybir as mybir
import concourse.tile as tile
from concourse import bacc
from concourse.bass_utils import run_bass_kernel_spmd

F32 = mybir.dt.float32
F32R = mybir.dt.float32r
BF16 = mybir.dt.bfloat16

B = 2
N = 2048          # sequence length
C = 1024          # model dim
HEADS_PER_CORE = 4
HD = 64           # head dim
SCALE = HD ** -0.5
NT = N // 128     # 16 row tiles
CT = C // 128     # 8 contraction tiles
QC = 4            # q-chunks of 512
QCS = N // QC     # 512
GROUPS = [[0, 1, 2, 3], [4, 5, 6, 7]]

_NC_CACHE = None


def build():
    nc = bacc.Bacc(None, target_bir_lowering=False, debug=False)

    xt_ext = nc.declare_dram_parameter("xt", [C, N], F32, isOutput=False)
    wqk_ext = nc.declare_dram_parameter("wqk", [C, 512], F32, isOutput=False)
    wv_ext = nc.declare_dram_parameter("wv", [C, 256], F32, isOutput=False)
    wpc_ext = nc.declare_dram_parameter("wpc", [C, 256], F32, isOutput=False)
    bc_ext = nc.declare_dram_parameter("bc", [128, 2], F32, isOutput=False)
    ones_ext = nc.declare_dram_parameter("ones64", [128, 64], F32, isOutput=False)
    out_ext = nc.declare_dram_parameter("out", [256, N], F32, isOutput=True)

    with tile.TileContext(nc) as tc:
        with (
            tc.tile_pool(name="weights", bufs=1) as wpool,
            tc.tile_pool(name="acts", bufs=1) as apool,
            tc.tile_pool(name="work", bufs=3) as work,
            tc.tile_pool(name="norm", bufs=2) as npool,
            tc.tile_pool(name="dram", bufs=2, space="DRAM") as dram,
        ):
            # ---- load inputs ----
            wqk_sb = wpool.tile([128, CT, 512], F32R, tag="wqk")
            wv_sb = wpool.tile([128, CT, 256], F32R, tag="wv")
            wp_sb = wpool.tile([128, CT, 256], F32R, tag="wp")
            bc_sb = wpool.tile([128, 2], F32, tag="bc")

            wqk_r = wqk_ext.ap().rearrange("(t p) n -> t p n", p=128)
            wv_r = wv_ext.ap().rearrange("(t p) n -> t p n", p=128)
            wpc_r = wpc_ext.ap().rearrange("(t p) n -> t p n", p=128)
            for ct in range(CT):
                nc.sync.dma_start(out=wqk_sb[:, ct, :], in_=wqk_r[ct].bitcast(F32R))
                nc.sync.dma_start(out=wv_sb[:, ct, :], in_=wv_r[ct].bitcast(F32R))
            for t in range(CT):
                nc.sync.dma_start(out=wp_sb[:, t, :], in_=wpc_r[t].bitcast(F32R))
            nc.sync.dma_start(out=bc_sb[:, :], in_=bc_ext[:, :])

            # ---- phase A: qkT = wqk.T @ xt   [512, 2048], v = xt.T @ wv [2048, 256+ones] ----
            qk_sb = apool.tile([128, 4, N], F32R, tag="qk")
            v_sb = apool.tile([128, NT, HEADS_PER_CORE, HD + 1], F32R, tag="v")
            # ones column for the row-sum trick (memset can't write f32r tiles;
            # DMA from a host constant instead)
            nc.sync.dma_start(
                out=v_sb[:, :, :, HD:HD + 1],
                in_=ones_ext.ap().rearrange("p (a b c) -> p a b c", a=NT, b=HEADS_PER_CORE).bitcast(F32R),
            )
            with (
                tc.tile_pool(name="xtpool", bufs=1) as xtpool,
                tc.tile_pool(name="psA", bufs=4, space="PSUM") as psA_pool,
            ):
                xt_sb = xtpool.tile([128, CT, N], F32R, tag="xt")
                xt_r = xt_ext.ap().rearrange("(t p) n -> t p n", p=128)
                for ct in range(CT):
                    nc.sync.dma_start(out=xt_sb[:, ct, :], in_=xt_r[ct].bitcast(F32R))
                for m in range(4):
                    for qn in range(QC):
                        psA = psA_pool.tile([128, QCS], F32, tag="psA")
                        for ct in range(CT):
                            nc.tensor.matmul(
                                psA[:, :],
                                wqk_sb[:, ct, m * 128:(m + 1) * 128],
                                xt_sb[:, ct, qn * QCS:(qn + 1) * QCS],
                                start=(ct == 0), stop=(ct == CT - 1),
                            )
                        nc.vector.tensor_copy(qk_sb[:, m, qn * QCS:(qn + 1) * QCS], psA[:, :])
                for rt in range(NT):
                    psV = psA_pool.tile([128, 256], F32, tag="psV")
                    for ct in range(CT):
                        nc.tensor.matmul(
                            psV[:, :],
                            xt_sb[:, ct, rt * 128:(rt + 1) * 128],
                            wv_sb[:, ct, :],
                            start=(ct == 0), stop=(ct == CT - 1),
                        )
                    nc.vector.tensor_copy(
                        v_sb[:, rt, :, 0:HD],
                        psV[:, :].rearrange("p (h e) -> p h e", h=HEADS_PER_CORE),
                    )

            # ---- phases B/C/D per q-chunk ----
            with (
                tc.tile_pool(name="ofpool", bufs=2) as ofpool,
                tc.tile_pool(name="psS", bufs=2, space="PSUM") as psS_pool,
                tc.tile_pool(name="psO", bufs=3, space="PSUM") as psO_pool,
                tc.tile_pool(name="psP", bufs=1, space="PSUM") as psP_pool,
            ):
                for qc in range(QC):
                    qsl = slice(qc * QCS, (qc + 1) * QCS)
                    on_sb = npool.tile([128, 2, QCS], F32R, tag="on")
                    for pr in range(2):
                        psO_e = psO_pool.tile([65, QCS], F32, tag="psO")
                        psO_o = psO_pool.tile([65, QCS], F32, tag="psO")
                        for kt in range(NT):
                            ksl = slice(kt * 128, (kt + 1) * 128)
                            psS = psS_pool.tile([128, 2 * QCS], F32, tag="psS")
                            nc.tensor.matmul(
                                psS[:, 0:QCS],
                                qk_sb[0:64, 2 + pr, ksl],
                                qk_sb[0:64, pr, qsl],
                                start=True, stop=True,
                            )
                            nc.tensor.matmul(
                                psS[:, QCS:2 * QCS],
                                qk_sb[64:128, 2 + pr, ksl],
                                qk_sb[64:128, pr, qsl],
                                start=True, stop=True,
                            )
                            expt = work.tile([128, 2 * QCS], F32R, tag="expt")
                            nc.scalar.activation(
                                expt[:, :], psS[:, :],
                                mybir.ActivationFunctionType.Exp,
                                bias=0.0, scale=SCALE,
                            )
                            nc.tensor.matmul(
                                psO_e[:, :],
                                v_sb[:, kt, 2 * pr, :],
                                expt[:, 0:QCS],
                                start=(kt == 0), stop=(kt == NT - 1),
                            )
                            nc.tensor.matmul(
                                psO_o[:, :],
                                v_sb[:, kt, 2 * pr + 1, :],
                                expt[:, QCS:2 * QCS],
                                start=(kt == 0), stop=(kt == NT - 1),
                            )
                        # normalize: o / rowsum (rowsum = psO[64], per q position).
                        # Reciprocal on a [1, N] row costs ~6.4ns/elem on one DVE
                        # lane; transpose the pair's 1024 sums to [128, 8] via a
                        # DRAM bounce so all lanes work, then broadcast-read back.
                        sums = npool.tile([65, 2 * QCS], F32, tag="sums")
                        nc.vector.tensor_copy(sums[64:65, 0:QCS], psO_e[64:65, :])
                        nc.vector.tensor_copy(sums[64:65, QCS:2 * QCS], psO_o[64:65, :])
                        row_dram = dram.tile([1, 2 * QCS], F32, tag="row")
                        nc.sync.dma_start(out=row_dram[:, :], in_=sums[64:65, :])
                        rd = row_dram[:, :]
                        st = npool.tile([128, 8], F32, tag="st")
                        nc.sync.dma_start(out=st[:, :], in_=bass.AP(
                            tensor=rd.tensor, offset=rd.offset, ap=[[1, 128], [128, 8]]))
                        rt = npool.tile([128, 8], F32, tag="rt")
                        nc.vector.reciprocal(rt[:, :], st[:, :])
                        rt_dram = dram.tile([8, 128], F32, tag="rtd")
                        rtd = rt_dram[:, :]
                        nc.sync.dma_start(
                            out=bass.AP(tensor=rtd.tensor, offset=rtd.offset,
                                        ap=[[1, 128], [128, 8]]),
                            in_=rt[:, :],
                        )
                        for hh, psO in ((0, psO_e), (1, psO_o)):
                            bcast = npool.tile([64, QCS], F32, tag="bcast")
                            nc.sync.dma_start(
                                out=bcast[:, :],
                                in_=bass.AP(tensor=rtd.tensor, offset=rtd.offset + hh * 4 * 128,
                                            ap=[[0, 64], [1, QCS]]),
                            )
                            nc.vector.tensor_mul(
                                on_sb[hh * 64:(hh + 1) * 64, pr, :],
                                psO[0:64, :],
                                bcast[:, :],
                            )
                    # ---- all-gather attention outputs (512KB, mesh regime), then
                    # compute this core's output-column slice of the projection ----
                    og = dram.tile([256, QCS], F32, tag="og")
                    nc.sync.dma_start(out=og[0:128, :], in_=on_sb[:, 0, :].bitcast(F32))
                    nc.sync.dma_start(out=og[128:256, :], in_=on_sb[:, 1, :].bitcast(F32))
                    ag = dram.tile([C, QCS], F32, tag="ag")
                    nc.gpsimd.collective_compute(
                        "AllGather",
                        mybir.AluOpType.bypass,
                        replica_groups=GROUPS,
                        ins=[og.opt()],
                        outs=[ag.opt()],
                    )
                    of_sb = ofpool.tile([128, CT, QCS], F32R, tag="of")
                    ag_r = ag[:, :].rearrange("(t p) n -> t p n", p=128)
                    for t in range(CT):
                        nc.sync.dma_start(out=of_sb[:, t, :], in_=ag_r[t].bitcast(F32R))
                    for m2 in range(2):
                        psP = psP_pool.tile([128, QCS], F32, tag="psP")
                        for kt8 in range(CT):
                            nc.tensor.matmul(
                                psP[:, :],
                                wp_sb[:, kt8, m2 * 128:(m2 + 1) * 128],
                                of_sb[:, kt8, :],
                                start=(kt8 == 0), stop=(kt8 == CT - 1),
                            )
                        outsb = work.tile([128, QCS], F32, tag="outsb")
                        nc.vector.tensor_scalar_add(outsb[:, :], psP[:, :], bc_sb[:, m2:m2 + 1])
                        nc.sync.dma_start(out=out_ext[m2 * 128:(m2 + 1) * 128, qsl], in_=outsb[:, :])

    nc.compile()
    return nc


def _get_nc():
    global _NC_CACHE
    if _NC_CACHE is None:
        _NC_CACHE = build()
    return _NC_CACHE


def shard_inputs(x, w_qkv, w_proj, b_proj):
    x = np.asarray(x, dtype=np.float32)
    w_qkv = np.asarray(w_qkv, dtype=np.float32)
    w_proj = np.asarray(w_proj, dtype=np.float32)
    b_proj = np.asarray(b_proj, dtype=np.float32)
    in_maps = []
    for core in range(8):
        b, g = divmod(core, 4)
        cs = slice(g * 256, (g + 1) * 256)
        wqk = np.concatenate([w_qkv[:, 0 * C + g * 256:0 * C + (g + 1) * 256],
                              w_qkv[:, 1 * C + g * 256:1 * C + (g + 1) * 256]], axis=1)
        in_maps.append({
            "xt": np.ascontiguousarray(x[b].T),
            "wqk": np.ascontiguousarray(wqk),
            "wv": np.ascontiguousarray(w_qkv[:, 2 * C + g * 256:2 * C + (g + 1) * 256]),
            "wpc": np.ascontiguousarray(w_proj[:, cs]),
            "bc": np.ascontiguousarray(b_proj[cs].reshape(2, 128).T),
            "ones64": np.ones((128, 64), dtype=np.float32),
        })
    return in_maps


def assemble_output(results):
    outT = np.empty((B, C, N), dtype=np.float32)
    for core in range(8):
        b, g = divmod(core, 4)
        outT[b, g * 256:(g + 1) * 256, :] = results[core]["out"]
    return np.ascontiguousarray(outT.transpose(0, 2, 1))


def run_sharded(x, w_qkv, w_proj, b_proj, trace=False):
    nc = _get_nc()
    in_maps = shard_inputs(x, w_qkv, w_proj, b_proj)
    res = run_bass_kernel_spmd(nc, in_maps, core_ids=list(range(8)), trace=trace)
    return assemble_output(res.results), res.exec_time_ns


def kernel(x, w_qkv, w_proj, b_proj):
    out, _ = run_sharded(x, w_qkv, w_proj, b_proj, trace=False)
    return out


# revision 15
# speedup vs baseline: 1.4009x; 1.4009x over previous
"""Multi-head attention block (B=2, N=2048, C=1024, H=16, hd=64) on 8 TRN2 NeuronCores.

Sharding: data-parallel over batch (2 groups of 4 cores), tensor-parallel over
heads within each group (4 heads/core). Each core computes q/k/v for its heads,
attention, and a partial output projection; a ReduceScatter over the 4-core
group sums the partials, and the host reassembles the full [2, 2048, 1024]
output from the per-core shards.

Per-core layouts (everything transposed so the contraction dim sits on SBUF
partitions; the host pre-transposes x):
  xt   [1024, 2048]  x[b].T
  wqk  [1024, 512]   w_qkv columns for this core's q (256) ++ k (256)
  wv   [1024, 256]   w_qkv columns for this core's v
  wpb  [256, 1024]   w_proj rows for this core's heads
  bq   [128, 8]      b_proj/4, bq[p, m] = b_proj[m*128+p]/4
  out  [256, 2048]   rows g*256:(g+1)*256 of (x[b] @ ... ).T after RS
"""
import sys

if '/opt/trn_rl_repo' not in sys.path:
    sys.path.insert(0, '/opt/trn_rl_repo')

import numpy as np

import concourse.bass as bass
import concourse.mybir as mybir
import concourse.tile as tile
from concourse import bacc
from concourse.bass_utils import run_bass_kernel_spmd

F32 = mybir.dt.float32
F32R = mybir.dt.float32r
BF16 = mybir.dt.bfloat16
F16 = mybir.dt.float16

B = 2
N = 2048          # sequence length
C = 1024          # model dim
HEADS_PER_CORE = 4
HD = 64           # head dim
SCALE = HD ** -0.5
NT = N // 128     # 16 row tiles
CT = C // 128     # 8 contraction tiles
QC = 4            # q-chunks of 512
QCS = N // QC     # 512
GROUPS = [[0, 1, 2, 3], [4, 5, 6, 7]]

_NC_CACHE = None


def build():
    nc = bacc.Bacc(None, target_bir_lowering=False, debug=False)

    xt_ext = nc.declare_dram_parameter("xt", [C, N], F32, isOutput=False)
    wqk_ext = nc.declare_dram_parameter("wqk", [C, 512], F32, isOutput=False)
    wv_ext = nc.declare_dram_parameter("wv", [C, 256], F32, isOutput=False)
    wpc_ext = nc.declare_dram_parameter("wpc", [C, 256], F32, isOutput=False)
    bc_ext = nc.declare_dram_parameter("bc", [128, 2], F32, isOutput=False)
    ones_ext = nc.declare_dram_parameter("ones64", [128, 64], F32, isOutput=False)
    out_ext = nc.declare_dram_parameter("out", [256, N], F32, isOutput=True)

    with tile.TileContext(nc) as tc:
        with (
            tc.tile_pool(name="weights", bufs=1) as wpool,
            tc.tile_pool(name="acts", bufs=1) as apool,
            tc.tile_pool(name="work", bufs=3) as work,
            tc.tile_pool(name="norm", bufs=2) as npool,
            tc.tile_pool(name="dram", bufs=2, space="DRAM") as dram,
        ):
            # ---- load inputs ----
            wqk_sb = wpool.tile([128, CT, 512], F16, tag="wqk")
            wv_sb = wpool.tile([128, CT, 256], F16, tag="wv")
            wp_sb = wpool.tile([128, CT, 256], F16, tag="wp")
            bc_sb = wpool.tile([128, 2], F32, tag="bc")

            wqk_r = wqk_ext.ap().rearrange("(t p) n -> t p n", p=128)
            wv_r = wv_ext.ap().rearrange("(t p) n -> t p n", p=128)
            wpc_r = wpc_ext.ap().rearrange("(t p) n -> t p n", p=128)
            for ct in range(CT):
                nc.gpsimd.dma_start(out=wqk_sb[:, ct, :], in_=wqk_r[ct])
                nc.gpsimd.dma_start(out=wv_sb[:, ct, :], in_=wv_r[ct])
            for t in range(CT):
                nc.gpsimd.dma_start(out=wp_sb[:, t, :], in_=wpc_r[t])
            nc.sync.dma_start(out=bc_sb[:, :], in_=bc_ext[:, :])

            # ---- phase A: qkT = wqk.T @ xt   [512, 2048], v = xt.T @ wv [2048, 256+ones] ----
            qk_sb = apool.tile([128, 4, N], F16, tag="qk")
            v_sb = apool.tile([128, NT, HEADS_PER_CORE, HD + 1], F16, tag="v")
            # ones column for the row-sum trick (memset can't write f32r tiles;
            # DMA from a host constant instead)
            nc.gpsimd.dma_start(
                out=v_sb[:, :, :, HD:HD + 1],
                in_=ones_ext.ap().rearrange("p (a b c) -> p a b c", a=NT, b=HEADS_PER_CORE),
            )
            with (
                tc.tile_pool(name="xtpool", bufs=1) as xtpool,
                tc.tile_pool(name="psA", bufs=4, space="PSUM") as psA_pool,
            ):
                xt_sb = xtpool.tile([128, CT, N], F16, tag="xt")
                xt_r = xt_ext.ap().rearrange("(t p) n -> t p n", p=128)
                for ct in range(CT):
                    nc.gpsimd.dma_start(out=xt_sb[:, ct, :], in_=xt_r[ct])
                for m in range(4):
                    for qn in range(QC):
                        psA = psA_pool.tile([128, QCS], F32, tag="psA")
                        for ct in range(CT):
                            nc.tensor.matmul(
                                psA[:, :],
                                wqk_sb[:, ct, m * 128:(m + 1) * 128],
                                xt_sb[:, ct, qn * QCS:(qn + 1) * QCS],
                                start=(ct == 0), stop=(ct == CT - 1),
                            )
                        nc.vector.tensor_copy(qk_sb[:, m, qn * QCS:(qn + 1) * QCS], psA[:, :])
                for rt in range(NT):
                    psV = psA_pool.tile([128, 256], F32, tag="psV")
                    for ct in range(CT):
                        nc.tensor.matmul(
                            psV[:, :],
                            xt_sb[:, ct, rt * 128:(rt + 1) * 128],
                            wv_sb[:, ct, :],
                            start=(ct == 0), stop=(ct == CT - 1),
                        )
                    nc.vector.tensor_copy(
                        v_sb[:, rt, :, 0:HD],
                        psV[:, :].rearrange("p (h e) -> p h e", h=HEADS_PER_CORE),
                    )

            # ---- phases B/C/D per q-chunk ----
            with (
                tc.tile_pool(name="ofpool", bufs=2) as ofpool,
                tc.tile_pool(name="psS", bufs=2, space="PSUM") as psS_pool,
                tc.tile_pool(name="psO", bufs=3, space="PSUM") as psO_pool,
                tc.tile_pool(name="psP", bufs=1, space="PSUM") as psP_pool,
            ):
                for qc in range(QC):
                    qsl = slice(qc * QCS, (qc + 1) * QCS)
                    on_sb = npool.tile([128, 2, QCS], F32, tag="on")
                    for pr in range(2):
                        psO_e = psO_pool.tile([65, QCS], F32, tag="psO")
                        psO_o = psO_pool.tile([65, QCS], F32, tag="psO")
                        for kt in range(NT):
                            ksl = slice(kt * 128, (kt + 1) * 128)
                            psS = psS_pool.tile([128, 2 * QCS], F32, tag="psS")
                            nc.tensor.matmul(
                                psS[:, 0:QCS],
                                qk_sb[0:64, 2 + pr, ksl],
                                qk_sb[0:64, pr, qsl],
                                start=True, stop=True,
                            )
                            nc.tensor.matmul(
                                psS[:, QCS:2 * QCS],
                                qk_sb[64:128, 2 + pr, ksl],
                                qk_sb[64:128, pr, qsl],
                                start=True, stop=True,
                            )
                            expt = work.tile([128, 2 * QCS], F16, tag="expt")
                            nc.scalar.activation(
                                expt[:, :], psS[:, :],
                                mybir.ActivationFunctionType.Exp,
                                bias=0.0, scale=SCALE,
                            )
                            nc.tensor.matmul(
                                psO_e[:, :],
                                v_sb[:, kt, 2 * pr, :],
                                expt[:, 0:QCS],
                                start=(kt == 0), stop=(kt == NT - 1),
                            )
                            nc.tensor.matmul(
                                psO_o[:, :],
                                v_sb[:, kt, 2 * pr + 1, :],
                                expt[:, QCS:2 * QCS],
                                start=(kt == 0), stop=(kt == NT - 1),
                            )
                        # normalize: o / rowsum  (rowsum is psO[64], per q position)
                        for hh, psO in ((0, psO_e), (1, psO_o)):
                            recip = npool.tile([65, QCS], F32, tag="recip")
                            nc.vector.reciprocal(recip[64:65, :], psO[64:65, :])
                            row_dram = dram.tile([1, QCS], F32, tag="row")
                            nc.sync.dma_start(out=row_dram[:, :], in_=recip[64:65, :])
                            rd = row_dram[:, :]
                            bcast_src = bass.AP(
                                tensor=rd.tensor, offset=rd.offset,
                                ap=[[0, 64]] + list(rd.ap[1:]),
                            )
                            bcast = npool.tile([64, QCS], F32, tag="bcast")
                            nc.sync.dma_start(out=bcast[:, :], in_=bcast_src)
                            nc.vector.tensor_mul(
                                on_sb[hh * 64:(hh + 1) * 64, pr, :],
                                psO[0:64, :],
                                bcast[:, :],
                            )
                    # ---- all-gather attention outputs (512KB, mesh regime), then
                    # compute this core's output-column slice of the projection ----
                    og = dram.tile([256, QCS], F32, tag="og")
                    nc.sync.dma_start(out=og[0:128, :], in_=on_sb[:, 0, :])
                    nc.sync.dma_start(out=og[128:256, :], in_=on_sb[:, 1, :])
                    ag = dram.tile([C, QCS], F32, tag="ag")
                    nc.gpsimd.collective_compute(
                        "AllGather",
                        mybir.AluOpType.bypass,
                        replica_groups=GROUPS,
                        ins=[og.opt()],
                        outs=[ag.opt()],
                    )
                    of_sb = ofpool.tile([128, CT, QCS], F16, tag="of")
                    ag_r = ag[:, :].rearrange("(t p) n -> t p n", p=128)
                    for t in range(CT):
                        nc.gpsimd.dma_start(out=of_sb[:, t, :], in_=ag_r[t])
                    for m2 in range(2):
                        psP = psP_pool.tile([128, QCS], F32, tag="psP")
                        for kt8 in range(CT):
                            nc.tensor.matmul(
                                psP[:, :],
                                wp_sb[:, kt8, m2 * 128:(m2 + 1) * 128],
                                of_sb[:, kt8, :],
                                start=(kt8 == 0), stop=(kt8 == CT - 1),
                            )
                        outsb = work.tile([128, QCS], F32, tag="outsb")
                        nc.vector.tensor_scalar_add(outsb[:, :], psP[:, :], bc_sb[:, m2:m2 + 1])
                        nc.sync.dma_start(out=out_ext[m2 * 128:(m2 + 1) * 128, qsl], in_=outsb[:, :])

    nc.compile()
    return nc


def _get_nc():
    global _NC_CACHE
    if _NC_CACHE is None:
        _NC_CACHE = build()
    return _NC_CACHE


def shard_inputs(x, w_qkv, w_proj, b_proj):
    x = np.asarray(x, dtype=np.float32)
    w_qkv = np.asarray(w_qkv, dtype=np.float32)
    w_proj = np.asarray(w_proj, dtype=np.float32)
    b_proj = np.asarray(b_proj, dtype=np.float32)
    in_maps = []
    for core in range(8):
        b, g = divmod(core, 4)
        cs = slice(g * 256, (g + 1) * 256)
        wqk = np.concatenate([w_qkv[:, 0 * C + g * 256:0 * C + (g + 1) * 256],
                              w_qkv[:, 1 * C + g * 256:1 * C + (g + 1) * 256]], axis=1)
        in_maps.append({
            "xt": np.ascontiguousarray(x[b].T),
            "wqk": np.ascontiguousarray(wqk),
            "wv": np.ascontiguousarray(w_qkv[:, 2 * C + g * 256:2 * C + (g + 1) * 256]),
            "wpc": np.ascontiguousarray(w_proj[:, cs]),
            "bc": np.ascontiguousarray(b_proj[cs].reshape(2, 128).T),
            "ones64": np.ones((128, 64), dtype=np.float32),
        })
    return in_maps


def assemble_output(results):
    outT = np.empty((B, C, N), dtype=np.float32)
    for core in range(8):
        b, g = divmod(core, 4)
        outT[b, g * 256:(g + 1) * 256, :] = results[core]["out"]
    return np.ascontiguousarray(outT.transpose(0, 2, 1))


def run_sharded(x, w_qkv, w_proj, b_proj, trace=False):
    nc = _get_nc()
    in_maps = shard_inputs(x, w_qkv, w_proj, b_proj)
    res = run_bass_kernel_spmd(nc, in_maps, core_ids=list(range(8)), trace=trace)
    return assemble_output(res.results), res.exec_time_ns


def kernel(x, w_qkv, w_proj, b_proj):
    out, _ = run_sharded(x, w_qkv, w_proj, b_proj, trace=False)
    return out


# revision 17
# speedup vs baseline: 1.5121x; 1.0794x over previous
"""Multi-head attention block (B=2, N=2048, C=1024, H=16, hd=64) on 8 TRN2 NeuronCores.

Sharding: data-parallel over batch (2 groups of 4 cores), tensor-parallel over
heads within each group (4 heads/core). Each core computes q/k/v for its heads,
attention, and a partial output projection; a ReduceScatter over the 4-core
group sums the partials, and the host reassembles the full [2, 2048, 1024]
output from the per-core shards.

Per-core layouts (everything transposed so the contraction dim sits on SBUF
partitions; the host pre-transposes x):
  xt   [1024, 2048]  x[b].T
  wqk  [1024, 512]   w_qkv columns for this core's q (256) ++ k (256)
  wv   [1024, 256]   w_qkv columns for this core's v
  wpb  [256, 1024]   w_proj rows for this core's heads
  bq   [128, 8]      b_proj/4, bq[p, m] = b_proj[m*128+p]/4
  out  [256, 2048]   rows g*256:(g+1)*256 of (x[b] @ ... ).T after RS
"""
import sys

if '/opt/trn_rl_repo' not in sys.path:
    sys.path.insert(0, '/opt/trn_rl_repo')

import numpy as np

import concourse.bass as bass
import concourse.mybir as mybir
import concourse.tile as tile
from concourse import bacc
from concourse.bass_utils import run_bass_kernel_spmd

F32 = mybir.dt.float32
F32R = mybir.dt.float32r
BF16 = mybir.dt.bfloat16
F16 = mybir.dt.float16

B = 2
N = 2048          # sequence length
C = 1024          # model dim
HEADS_PER_CORE = 4
HD = 64           # head dim
SCALE = HD ** -0.5
NT = N // 128     # 16 row tiles
CT = C // 128     # 8 contraction tiles
QC = 4            # q-chunks of 512
QCS = N // QC     # 512
GROUPS = [[0, 1, 2, 3], [4, 5, 6, 7]]

_NC_CACHE = None


def build():
    nc = bacc.Bacc(None, target_bir_lowering=False, debug=False)

    xt_ext = nc.declare_dram_parameter("xt", [C, N], F32, isOutput=False)
    wqk_ext = nc.declare_dram_parameter("wqk", [C, 512], F32, isOutput=False)
    wv_ext = nc.declare_dram_parameter("wv", [C, 256], F32, isOutput=False)
    wpc_ext = nc.declare_dram_parameter("wpc", [C, 256], F32, isOutput=False)
    bc_ext = nc.declare_dram_parameter("bc", [128, 2], F32, isOutput=False)
    ones_ext = nc.declare_dram_parameter("ones64", [128, 64], F32, isOutput=False)
    zeros_ext = nc.declare_dram_parameter("zeros63", [128, 63], F32, isOutput=False)
    out_ext = nc.declare_dram_parameter("out", [256, N], F32, isOutput=True)

    with tile.TileContext(nc) as tc:
        with (
            tc.tile_pool(name="weights", bufs=1) as wpool,
            tc.tile_pool(name="acts", bufs=1) as apool,
            tc.tile_pool(name="work", bufs=3) as work,
            tc.tile_pool(name="norm", bufs=2) as npool,
            tc.tile_pool(name="dram", bufs=2, space="DRAM") as dram,
        ):
            # ---- load inputs ----
            wqk_sb = wpool.tile([128, CT, 512], F16, tag="wqk")
            wv_sb = wpool.tile([128, CT, 256], F16, tag="wv")
            wp_sb = wpool.tile([128, CT, 256], F16, tag="wp")
            bc_sb = wpool.tile([128, 2], F32, tag="bc")

            wqk_r = wqk_ext.ap().rearrange("(t p) n -> t p n", p=128)
            wv_r = wv_ext.ap().rearrange("(t p) n -> t p n", p=128)
            wpc_r = wpc_ext.ap().rearrange("(t p) n -> t p n", p=128)
            for ct in range(CT):
                nc.gpsimd.dma_start(out=wqk_sb[:, ct, :], in_=wqk_r[ct])
                nc.gpsimd.dma_start(out=wv_sb[:, ct, :], in_=wv_r[ct])
            for t in range(CT):
                nc.gpsimd.dma_start(out=wp_sb[:, t, :], in_=wpc_r[t])
            nc.sync.dma_start(out=bc_sb[:, :], in_=bc_ext[:, :])

            # ---- phase A: qkT = wqk.T @ xt   [512, 2048], v = xt.T @ wv [2048, 256+ones] ----
            qk_sb = apool.tile([128, 4, N], F16, tag="qk")
            v_sb = apool.tile([128, NT, HEADS_PER_CORE, 128], F16, tag="v")
            # ones column for the row-sum trick (memset can't write f32r tiles;
            # DMA from a host constant instead)
            nc.gpsimd.dma_start(
                out=v_sb[:, :, :, HD:HD + 1],
                in_=ones_ext.ap().rearrange("p (a b c) -> p a b c", a=NT, b=HEADS_PER_CORE),
            )
            # cols HD+1..127 of each v block are never-read pad (FWL needs a
            # 128-col stationary); zero them so no stale NaNs enter PSUM
            nc.gpsimd.dma_start(
                out=v_sb[:, :, :, HD + 1:128],
                in_=bass.AP(tensor=zeros_ext.ap().tensor, offset=0,
                            ap=[[63, 128], [0, NT * HEADS_PER_CORE], [1, 63]]),
            )
            with (
                tc.tile_pool(name="xtpool", bufs=1) as xtpool,
                tc.tile_pool(name="psA", bufs=4, space="PSUM") as psA_pool,
            ):
                xt_sb = xtpool.tile([128, CT, N], F16, tag="xt")
                xt_r = xt_ext.ap().rearrange("(t p) n -> t p n", p=128)
                for ct in range(CT):
                    nc.gpsimd.dma_start(out=xt_sb[:, ct, :], in_=xt_r[ct])
                for m in range(4):
                    for qn in range(QC):
                        psA = psA_pool.tile([128, QCS], F32, tag="psA")
                        for ct in range(CT):
                            nc.tensor.matmul(
                                psA[:, :],
                                wqk_sb[:, ct, m * 128:(m + 1) * 128],
                                xt_sb[:, ct, qn * QCS:(qn + 1) * QCS],
                                start=(ct == 0), stop=(ct == CT - 1),
                            )
                        nc.vector.tensor_copy(qk_sb[:, m, qn * QCS:(qn + 1) * QCS], psA[:, :])
                for rt in range(NT):
                    psV = psA_pool.tile([128, 256], F32, tag="psV")
                    for ct in range(CT):
                        nc.tensor.matmul(
                            psV[:, :],
                            xt_sb[:, ct, rt * 128:(rt + 1) * 128],
                            wv_sb[:, ct, :],
                            start=(ct == 0), stop=(ct == CT - 1),
                        )
                    nc.vector.tensor_copy(
                        v_sb[:, rt, :, 0:HD],
                        psV[:, :].rearrange("p (h e) -> p h e", h=HEADS_PER_CORE),
                    )

            # ---- phases B/C/D per q-chunk ----
            with (
                tc.tile_pool(name="ofpool", bufs=2) as ofpool,
                tc.tile_pool(name="psS", bufs=2, space="PSUM") as psS_pool,
                tc.tile_pool(name="psO", bufs=3, space="PSUM") as psO_pool,
                tc.tile_pool(name="psP", bufs=1, space="PSUM") as psP_pool,
            ):
                for qc in range(QC):
                    qsl = slice(qc * QCS, (qc + 1) * QCS)
                    on_sb = npool.tile([128, 2, QCS], F16, tag="on")
                    for pr in range(2):
                        psO_e = psO_pool.tile([128, QCS], F32, tag="psO")
                        psO_o = psO_pool.tile([128, QCS], F32, tag="psO")
                        for kt in range(NT):
                            ksl = slice(kt * 128, (kt + 1) * 128)
                            psS = psS_pool.tile([128, 2 * QCS], F32, tag="psS")
                            nc.tensor.matmul(
                                psS[:, 0:QCS],
                                qk_sb[0:64, 2 + pr, ksl],
                                qk_sb[0:64, pr, qsl],
                                start=True, stop=True,
                            )
                            nc.tensor.matmul(
                                psS[:, QCS:2 * QCS],
                                qk_sb[64:128, 2 + pr, ksl],
                                qk_sb[64:128, pr, qsl],
                                start=True, stop=True,
                            )
                            expt = work.tile([128, 2 * QCS], F16, tag="expt")
                            nc.scalar.activation(
                                expt[:, :], psS[:, :],
                                mybir.ActivationFunctionType.Exp,
                                bias=0.0, scale=SCALE,
                            )
                            nc.tensor.matmul(
                                psO_e[:, :],
                                v_sb[:, kt, 2 * pr, 0:128],
                                expt[:, 0:QCS],
                                start=(kt == 0), stop=(kt == NT - 1),
                            )
                            nc.tensor.matmul(
                                psO_o[:, :],
                                v_sb[:, kt, 2 * pr + 1, 0:128],
                                expt[:, QCS:2 * QCS],
                                start=(kt == 0), stop=(kt == NT - 1),
                            )
                        # normalize: o / rowsum  (rowsum is psO[64], per q position)
                        for hh, psO in ((0, psO_e), (1, psO_o)):
                            recip = npool.tile([65, QCS], F32, tag="recip")
                            nc.vector.reciprocal(recip[64:65, :], psO[64:65, :])
                            row_dram = dram.tile([1, QCS], F32, tag="row")
                            nc.sync.dma_start(out=row_dram[:, :], in_=recip[64:65, :])
                            rd = row_dram[:, :]
                            bcast_src = bass.AP(
                                tensor=rd.tensor, offset=rd.offset,
                                ap=[[0, 64]] + list(rd.ap[1:]),
                            )
                            bcast = npool.tile([64, QCS], F32, tag="bcast")
                            nc.sync.dma_start(out=bcast[:, :], in_=bcast_src)
                            nc.vector.tensor_mul(
                                on_sb[hh * 64:(hh + 1) * 64, pr, :],
                                psO[0:64, :],
                                bcast[:, :],
                            )
                    # ---- all-gather attention outputs (512KB, mesh regime), then
                    # compute this core's output-column slice of the projection ----
                    og = dram.tile([256, QCS], F16, tag="og")
                    nc.sync.dma_start(out=og[0:128, :], in_=on_sb[:, 0, :])
                    nc.sync.dma_start(out=og[128:256, :], in_=on_sb[:, 1, :])
                    ag = dram.tile([C, QCS], F16, tag="ag")
                    nc.gpsimd.collective_compute(
                        "AllGather",
                        mybir.AluOpType.bypass,
                        replica_groups=GROUPS,
                        ins=[og.opt()],
                        outs=[ag.opt()],
                    )
                    of_sb = ofpool.tile([128, CT, QCS], F16, tag="of")
                    ag_r = ag[:, :].rearrange("(t p) n -> t p n", p=128)
                    for t in range(CT):
                        nc.sync.dma_start(out=of_sb[:, t, :], in_=ag_r[t])
                    for m2 in range(2):
                        psP = psP_pool.tile([128, QCS], F32, tag="psP")
                        for kt8 in range(CT):
                            nc.tensor.matmul(
                                psP[:, :],
                                wp_sb[:, kt8, m2 * 128:(m2 + 1) * 128],
                                of_sb[:, kt8, :],
                                start=(kt8 == 0), stop=(kt8 == CT - 1),
                            )
                        outsb = work.tile([128, QCS], F32, tag="outsb")
                        nc.vector.tensor_scalar_add(outsb[:, :], psP[:, :], bc_sb[:, m2:m2 + 1])
                        nc.sync.dma_start(out=out_ext[m2 * 128:(m2 + 1) * 128, qsl], in_=outsb[:, :])

    nc.compile()
    return nc


def _get_nc():
    global _NC_CACHE
    if _NC_CACHE is None:
        _NC_CACHE = build()
    return _NC_CACHE


def shard_inputs(x, w_qkv, w_proj, b_proj):
    x = np.asarray(x, dtype=np.float32)
    w_qkv = np.asarray(w_qkv, dtype=np.float32)
    w_proj = np.asarray(w_proj, dtype=np.float32)
    b_proj = np.asarray(b_proj, dtype=np.float32)
    in_maps = []
    for core in range(8):
        b, g = divmod(core, 4)
        cs = slice(g * 256, (g + 1) * 256)
        wqk = np.concatenate([w_qkv[:, 0 * C + g * 256:0 * C + (g + 1) * 256],
                              w_qkv[:, 1 * C + g * 256:1 * C + (g + 1) * 256]], axis=1)
        in_maps.append({
            "xt": np.ascontiguousarray(x[b].T),
            "wqk": np.ascontiguousarray(wqk),
            "wv": np.ascontiguousarray(w_qkv[:, 2 * C + g * 256:2 * C + (g + 1) * 256]),
            "wpc": np.ascontiguousarray(w_proj[:, cs]),
            "bc": np.ascontiguousarray(b_proj[cs].reshape(2, 128).T),
            "ones64": np.ones((128, 64), dtype=np.float32),
            "zeros63": np.zeros((128, 63), dtype=np.float32),
        })
    return in_maps


def assemble_output(results):
    outT = np.empty((B, C, N), dtype=np.float32)
    for core in range(8):
        b, g = divmod(core, 4)
        outT[b, g * 256:(g + 1) * 256, :] = results[core]["out"]
    return np.ascontiguousarray(outT.transpose(0, 2, 1))


def run_sharded(x, w_qkv, w_proj, b_proj, trace=False):
    nc = _get_nc()
    in_maps = shard_inputs(x, w_qkv, w_proj, b_proj)
    res = run_bass_kernel_spmd(nc, in_maps, core_ids=list(range(8)), trace=trace)
    return assemble_output(res.results), res.exec_time_ns


def kernel(x, w_qkv, w_proj, b_proj):
    out, _ = run_sharded(x, w_qkv, w_proj, b_proj, trace=False)
    return out


# revision 18
# speedup vs baseline: 1.5978x; 1.0567x over previous
"""Multi-head attention block (B=2, N=2048, C=1024, H=16, hd=64) on 8 TRN2 NeuronCores.

Sharding: data-parallel over batch (2 groups of 4 cores), tensor-parallel over
heads within each group (4 heads/core). Each core computes q/k/v for its heads,
attention, and a partial output projection; a ReduceScatter over the 4-core
group sums the partials, and the host reassembles the full [2, 2048, 1024]
output from the per-core shards.

Per-core layouts (everything transposed so the contraction dim sits on SBUF
partitions; the host pre-transposes x):
  xt   [1024, 2048]  x[b].T
  wqk  [1024, 512]   w_qkv columns for this core's q (256) ++ k (256)
  wv   [1024, 256]   w_qkv columns for this core's v
  wpb  [256, 1024]   w_proj rows for this core's heads
  bq   [128, 8]      b_proj/4, bq[p, m] = b_proj[m*128+p]/4
  out  [256, 2048]   rows g*256:(g+1)*256 of (x[b] @ ... ).T after RS
"""
import sys

if '/opt/trn_rl_repo' not in sys.path:
    sys.path.insert(0, '/opt/trn_rl_repo')

import numpy as np

import concourse.bass as bass
import concourse.mybir as mybir
import concourse.tile as tile
from concourse import bacc
from concourse.bass_utils import run_bass_kernel_spmd

F32 = mybir.dt.float32
F32R = mybir.dt.float32r
BF16 = mybir.dt.bfloat16
F16 = mybir.dt.float16

B = 2
N = 2048          # sequence length
C = 1024          # model dim
HEADS_PER_CORE = 4
HD = 64           # head dim
SCALE = HD ** -0.5
NT = N // 128     # 16 row tiles
CT = C // 128     # 8 contraction tiles
QC = 4            # q-chunks of 512
QCS = N // QC     # 512
GROUPS = [[0, 1, 2, 3], [4, 5, 6, 7]]

_NC_CACHE = None


def build():
    nc = bacc.Bacc(None, target_bir_lowering=False, debug=False)

    xt_ext = nc.declare_dram_parameter("xt", [C, N], F16, isOutput=False)
    wqk_ext = nc.declare_dram_parameter("wqk", [C, 512], F16, isOutput=False)
    wv_ext = nc.declare_dram_parameter("wv", [C, 256], F16, isOutput=False)
    wpc_ext = nc.declare_dram_parameter("wpc", [C, 256], F16, isOutput=False)
    bc_ext = nc.declare_dram_parameter("bc", [128, 2], F32, isOutput=False)
    ones_ext = nc.declare_dram_parameter("ones64", [128, 64], F16, isOutput=False)
    zeros_ext = nc.declare_dram_parameter("zeros63", [128, 63], F16, isOutput=False)
    out_ext = nc.declare_dram_parameter("out", [256, N], F32, isOutput=True)

    with tile.TileContext(nc) as tc:
        with (
            tc.tile_pool(name="weights", bufs=1) as wpool,
            tc.tile_pool(name="acts", bufs=1) as apool,
            tc.tile_pool(name="work", bufs=3) as work,
            tc.tile_pool(name="norm", bufs=2) as npool,
            tc.tile_pool(name="dram", bufs=2, space="DRAM") as dram,
        ):
            # ---- load inputs ----
            wqk_sb = wpool.tile([128, CT, 512], F16, tag="wqk")
            wv_sb = wpool.tile([128, CT, 256], F16, tag="wv")
            wp_sb = wpool.tile([128, CT, 256], F16, tag="wp")
            bc_sb = wpool.tile([128, 2], F32, tag="bc")

            wqk_r = wqk_ext.ap().rearrange("(t p) n -> t p n", p=128)
            wv_r = wv_ext.ap().rearrange("(t p) n -> t p n", p=128)
            wpc_r = wpc_ext.ap().rearrange("(t p) n -> t p n", p=128)
            for ct in range(CT):
                nc.sync.dma_start(out=wqk_sb[:, ct, :], in_=wqk_r[ct])
                nc.sync.dma_start(out=wv_sb[:, ct, :], in_=wv_r[ct])
            for t in range(CT):
                nc.sync.dma_start(out=wp_sb[:, t, :], in_=wpc_r[t])
            nc.sync.dma_start(out=bc_sb[:, :], in_=bc_ext[:, :])

            # ---- phase A: qkT = wqk.T @ xt   [512, 2048], v = xt.T @ wv [2048, 256+ones] ----
            qk_sb = apool.tile([128, 4, N], F16, tag="qk")
            v_sb = apool.tile([128, NT, HEADS_PER_CORE, 128], F16, tag="v")
            # ones column for the row-sum trick (memset can't write f32r tiles;
            # DMA from a host constant instead)
            nc.sync.dma_start(
                out=v_sb[:, :, :, HD:HD + 1],
                in_=ones_ext.ap().rearrange("p (a b c) -> p a b c", a=NT, b=HEADS_PER_CORE),
            )
            # cols HD+1..127 of each v block are never-read pad (FWL needs a
            # 128-col stationary); zero them so no stale NaNs enter PSUM
            nc.sync.dma_start(
                out=v_sb[:, :, :, HD + 1:128],
                in_=bass.AP(tensor=zeros_ext.ap().tensor, offset=0,
                            ap=[[63, 128], [0, NT * HEADS_PER_CORE], [1, 63]]),
            )
            with (
                tc.tile_pool(name="xtpool", bufs=1) as xtpool,
                tc.tile_pool(name="psA", bufs=4, space="PSUM") as psA_pool,
            ):
                xt_sb = xtpool.tile([128, CT, N], F16, tag="xt")
                xt_r = xt_ext.ap().rearrange("(t p) n -> t p n", p=128)
                for ct in range(CT):
                    nc.sync.dma_start(out=xt_sb[:, ct, :], in_=xt_r[ct])
                for m in range(4):
                    for qn in range(QC):
                        psA = psA_pool.tile([128, QCS], F32, tag="psA")
                        for ct in range(CT):
                            nc.tensor.matmul(
                                psA[:, :],
                                wqk_sb[:, ct, m * 128:(m + 1) * 128],
                                xt_sb[:, ct, qn * QCS:(qn + 1) * QCS],
                                start=(ct == 0), stop=(ct == CT - 1),
                            )
                        nc.vector.tensor_copy(qk_sb[:, m, qn * QCS:(qn + 1) * QCS], psA[:, :])
                for rt in range(NT):
                    psV = psA_pool.tile([128, 256], F32, tag="psV")
                    for ct in range(CT):
                        nc.tensor.matmul(
                            psV[:, :],
                            xt_sb[:, ct, rt * 128:(rt + 1) * 128],
                            wv_sb[:, ct, :],
                            start=(ct == 0), stop=(ct == CT - 1),
                        )
                    nc.vector.tensor_copy(
                        v_sb[:, rt, :, 0:HD],
                        psV[:, :].rearrange("p (h e) -> p h e", h=HEADS_PER_CORE),
                    )

            # ---- phases B/C/D per q-chunk ----
            with (
                tc.tile_pool(name="ofpool", bufs=2) as ofpool,
                tc.tile_pool(name="psS", bufs=2, space="PSUM") as psS_pool,
                tc.tile_pool(name="psO", bufs=3, space="PSUM") as psO_pool,
                tc.tile_pool(name="psP", bufs=1, space="PSUM") as psP_pool,
            ):
                for qc in range(QC):
                    qsl = slice(qc * QCS, (qc + 1) * QCS)
                    on_sb = npool.tile([128, 2, QCS], F16, tag="on")
                    for pr in range(2):
                        psO_e = psO_pool.tile([128, QCS], F32, tag="psO")
                        psO_o = psO_pool.tile([128, QCS], F32, tag="psO")
                        for kt in range(NT):
                            ksl = slice(kt * 128, (kt + 1) * 128)
                            psS = psS_pool.tile([128, 2 * QCS], F32, tag="psS")
                            nc.tensor.matmul(
                                psS[:, 0:QCS],
                                qk_sb[0:64, 2 + pr, ksl],
                                qk_sb[0:64, pr, qsl],
                                start=True, stop=True,
                            )
                            nc.tensor.matmul(
                                psS[:, QCS:2 * QCS],
                                qk_sb[64:128, 2 + pr, ksl],
                                qk_sb[64:128, pr, qsl],
                                start=True, stop=True,
                            )
                            expt = work.tile([128, 2 * QCS], F16, tag="expt")
                            nc.scalar.activation(
                                expt[:, :], psS[:, :],
                                mybir.ActivationFunctionType.Exp,
                                bias=0.0, scale=SCALE,
                            )
                            nc.tensor.matmul(
                                psO_e[:, :],
                                v_sb[:, kt, 2 * pr, 0:128],
                                expt[:, 0:QCS],
                                start=(kt == 0), stop=(kt == NT - 1),
                            )
                            nc.tensor.matmul(
                                psO_o[:, :],
                                v_sb[:, kt, 2 * pr + 1, 0:128],
                                expt[:, QCS:2 * QCS],
                                start=(kt == 0), stop=(kt == NT - 1),
                            )
                        # normalize: o / rowsum  (rowsum is psO[64], per q position)
                        for hh, psO in ((0, psO_e), (1, psO_o)):
                            recip = npool.tile([65, QCS], F32, tag="recip")
                            nc.vector.reciprocal(recip[64:65, :], psO[64:65, :])
                            row_dram = dram.tile([1, QCS], F32, tag="row")
                            nc.sync.dma_start(out=row_dram[:, :], in_=recip[64:65, :])
                            rd = row_dram[:, :]
                            bcast_src = bass.AP(
                                tensor=rd.tensor, offset=rd.offset,
                                ap=[[0, 64]] + list(rd.ap[1:]),
                            )
                            bcast = npool.tile([64, QCS], F32, tag="bcast")
                            nc.sync.dma_start(out=bcast[:, :], in_=bcast_src)
                            nc.vector.tensor_mul(
                                on_sb[hh * 64:(hh + 1) * 64, pr, :],
                                psO[0:64, :],
                                bcast[:, :],
                            )
                    # ---- all-gather attention outputs (512KB, mesh regime), then
                    # compute this core's output-column slice of the projection ----
                    og = dram.tile([256, QCS], F16, tag="og")
                    nc.sync.dma_start(out=og[0:128, :], in_=on_sb[:, 0, :])
                    nc.sync.dma_start(out=og[128:256, :], in_=on_sb[:, 1, :])
                    ag = dram.tile([C, QCS], F16, tag="ag")
                    nc.gpsimd.collective_compute(
                        "AllGather",
                        mybir.AluOpType.bypass,
                        replica_groups=GROUPS,
                        ins=[og.opt()],
                        outs=[ag.opt()],
                    )
                    of_sb = ofpool.tile([128, CT, QCS], F16, tag="of")
                    ag_r = ag[:, :].rearrange("(t p) n -> t p n", p=128)
                    for t in range(CT):
                        nc.sync.dma_start(out=of_sb[:, t, :], in_=ag_r[t])
                    for m2 in range(2):
                        psP = psP_pool.tile([128, QCS], F32, tag="psP")
                        for kt8 in range(CT):
                            nc.tensor.matmul(
                                psP[:, :],
                                wp_sb[:, kt8, m2 * 128:(m2 + 1) * 128],
                                of_sb[:, kt8, :],
                                start=(kt8 == 0), stop=(kt8 == CT - 1),
                            )
                        outsb = work.tile([128, QCS], F32, tag="outsb")
                        nc.vector.tensor_scalar_add(outsb[:, :], psP[:, :], bc_sb[:, m2:m2 + 1])
                        nc.sync.dma_start(out=out_ext[m2 * 128:(m2 + 1) * 128, qsl], in_=outsb[:, :])

    nc.compile()
    return nc


def _get_nc():
    global _NC_CACHE
    if _NC_CACHE is None:
        _NC_CACHE = build()
    return _NC_CACHE


def shard_inputs(x, w_qkv, w_proj, b_proj):
    x = np.asarray(x, dtype=np.float32)
    w_qkv = np.asarray(w_qkv, dtype=np.float32)
    w_proj = np.asarray(w_proj, dtype=np.float32)
    b_proj = np.asarray(b_proj, dtype=np.float32)
    in_maps = []
    for core in range(8):
        b, g = divmod(core, 4)
        cs = slice(g * 256, (g + 1) * 256)
        wqk = np.concatenate([w_qkv[:, 0 * C + g * 256:0 * C + (g + 1) * 256],
                              w_qkv[:, 1 * C + g * 256:1 * C + (g + 1) * 256]], axis=1)
        in_maps.append({
            "xt": np.ascontiguousarray(x[b].T.astype(np.float16)),
            "wqk": np.ascontiguousarray(wqk.astype(np.float16)),
            "wv": np.ascontiguousarray(w_qkv[:, 2 * C + g * 256:2 * C + (g + 1) * 256].astype(np.float16)),
            "wpc": np.ascontiguousarray(w_proj[:, cs].astype(np.float16)),
            "bc": np.ascontiguousarray(b_proj[cs].reshape(2, 128).T),
            "ones64": np.ones((128, 64), dtype=np.float16),
            "zeros63": np.zeros((128, 63), dtype=np.float16),
        })
    return in_maps


def assemble_output(results):
    outT = np.empty((B, C, N), dtype=np.float32)
    for core in range(8):
        b, g = divmod(core, 4)
        outT[b, g * 256:(g + 1) * 256, :] = results[core]["out"]
    return np.ascontiguousarray(outT.transpose(0, 2, 1))


def run_sharded(x, w_qkv, w_proj, b_proj, trace=False):
    nc = _get_nc()
    in_maps = shard_inputs(x, w_qkv, w_proj, b_proj)
    res = run_bass_kernel_spmd(nc, in_maps, core_ids=list(range(8)), trace=trace)
    return assemble_output(res.results), res.exec_time_ns


def kernel(x, w_qkv, w_proj, b_proj):
    out, _ = run_sharded(x, w_qkv, w_proj, b_proj, trace=False)
    return out


# revision 21
# speedup vs baseline: 1.6166x; 1.0118x over previous
"""Multi-head attention block (B=2, N=2048, C=1024, H=16, hd=64) on 8 TRN2 NeuronCores.

Sharding: data-parallel over batch (2 groups of 4 cores), tensor-parallel over
heads within each group (4 heads/core). Each core computes q/k/v for its heads,
attention, and a partial output projection; a ReduceScatter over the 4-core
group sums the partials, and the host reassembles the full [2, 2048, 1024]
output from the per-core shards.

Per-core layouts (everything transposed so the contraction dim sits on SBUF
partitions; the host pre-transposes x):
  xt   [1024, 2048]  x[b].T
  wqk  [1024, 512]   w_qkv columns for this core's q (256) ++ k (256)
  wv   [1024, 256]   w_qkv columns for this core's v
  wpb  [256, 1024]   w_proj rows for this core's heads
  bq   [128, 8]      b_proj/4, bq[p, m] = b_proj[m*128+p]/4
  out  [256, 2048]   rows g*256:(g+1)*256 of (x[b] @ ... ).T after RS
"""
import sys

if '/opt/trn_rl_repo' not in sys.path:
    sys.path.insert(0, '/opt/trn_rl_repo')

import numpy as np

import concourse.bass as bass
import concourse.mybir as mybir
import concourse.tile as tile
from concourse import bacc
from concourse.bass_utils import run_bass_kernel_spmd

F32 = mybir.dt.float32
F32R = mybir.dt.float32r
BF16 = mybir.dt.bfloat16
F16 = mybir.dt.float16

B = 2
N = 2048          # sequence length
C = 1024          # model dim
HEADS_PER_CORE = 4
HD = 64           # head dim
SCALE = HD ** -0.5
NT = N // 128     # 16 row tiles
CT = C // 128     # 8 contraction tiles
QC = 4            # q-chunks of 512
QCS = N // QC     # 512
GROUPS = [[0, 1, 2, 3], [4, 5, 6, 7]]

_NC_CACHE = None


def build():
    nc = bacc.Bacc(None, target_bir_lowering=False, debug=False)

    xt_ext = nc.declare_dram_parameter("xt", [C, N], F16, isOutput=False)
    wqk_ext = nc.declare_dram_parameter("wqk", [C, 512], F16, isOutput=False)
    wv_ext = nc.declare_dram_parameter("wv", [C, 256], F16, isOutput=False)
    wpc_ext = nc.declare_dram_parameter("wpc", [C, 256], F16, isOutput=False)
    bc_ext = nc.declare_dram_parameter("bc", [128, 2], F32, isOutput=False)
    ones_ext = nc.declare_dram_parameter("ones64", [128, 64], F16, isOutput=False)
    zeros_ext = nc.declare_dram_parameter("zeros63", [128, 63], F16, isOutput=False)
    out_ext = nc.declare_dram_parameter("out", [256, N], F16, isOutput=True)

    with tile.TileContext(nc) as tc:
        with (
            tc.tile_pool(name="weights", bufs=1) as wpool,
            tc.tile_pool(name="acts", bufs=1) as apool,
            tc.tile_pool(name="work", bufs=4) as work,
            tc.tile_pool(name="norm", bufs=2) as npool,
            tc.tile_pool(name="dram", bufs=2, space="DRAM") as dram,
        ):
            # ---- load inputs ----
            wqk_sb = wpool.tile([128, CT, 512], F16, tag="wqk")
            wv_sb = wpool.tile([128, CT, 256], F16, tag="wv")
            wp_sb = wpool.tile([128, CT, 256], F16, tag="wp")
            bc_sb = wpool.tile([128, 2], F32, tag="bc")

            wqk_r = wqk_ext.ap().rearrange("(t p) n -> t p n", p=128)
            wv_r = wv_ext.ap().rearrange("(t p) n -> t p n", p=128)
            wpc_r = wpc_ext.ap().rearrange("(t p) n -> t p n", p=128)


            # ---- phase A: qkT = wqk.T @ xt   [512, 2048], v = xt.T @ wv [2048, 256+ones] ----
            qk_sb = apool.tile([128, 4, N], F16, tag="qk")
            v_sb = apool.tile([128, NT, HEADS_PER_CORE, 128], F16, tag="v")
            # ones column for the row-sum trick (memset can't write f32r tiles;
            # DMA from a host constant instead)
            nc.sync.dma_start(
                out=v_sb[:, :, :, HD:HD + 1],
                in_=ones_ext.ap().rearrange("p (a b c) -> p a b c", a=NT, b=HEADS_PER_CORE),
            )
            # cols HD+1..127 of each v block are never-read pad (FWL needs a
            # 128-col stationary); zero them so no stale NaNs enter PSUM
            nc.sync.dma_start(
                out=v_sb[:, :, :, HD + 1:128],
                in_=bass.AP(tensor=zeros_ext.ap().tensor, offset=0,
                            ap=[[63, 128], [0, NT * HEADS_PER_CORE], [1, 63]]),
            )
            with (
                tc.tile_pool(name="xtpool", bufs=1) as xtpool,
                tc.tile_pool(name="psA", bufs=4, space="PSUM") as psA_pool,
            ):
                xt_sb = xtpool.tile([128, CT, N], F16, tag="xt")
                xt_r = xt_ext.ap().rearrange("(t p) n -> t p n", p=128)
                for ct in range(CT):
                    nc.sync.dma_start(out=xt_sb[:, ct, :], in_=xt_r[ct])
                    nc.sync.dma_start(out=wqk_sb[:, ct, :], in_=wqk_r[ct])
                    nc.sync.dma_start(out=wv_sb[:, ct, :], in_=wv_r[ct])
                    nc.sync.dma_start(out=wp_sb[:, ct, :], in_=wpc_r[ct])
                nc.sync.dma_start(out=bc_sb[:, :], in_=bc_ext[:, :])
                for m in range(4):
                    psAs = [psA_pool.tile([128, QCS], F32, tag="psA", name=f"psA_{m}_{i}") for i in range(QC)]
                    for ct in range(CT):
                        for qn in range(QC):
                            nc.tensor.matmul(
                                psAs[qn][:, :],
                                wqk_sb[:, ct, m * 128:(m + 1) * 128],
                                xt_sb[:, ct, qn * QCS:(qn + 1) * QCS],
                                start=(ct == 0), stop=(ct == CT - 1),
                            )
                    for qn in range(QC):
                        nc.vector.tensor_copy(qk_sb[:, m, qn * QCS:(qn + 1) * QCS], psAs[qn][:, :])
                for rtc in range(NT // 4):
                    psVs = [psA_pool.tile([128, 256], F32, tag="psA", name=f"psV_{rtc}_{i}") for i in range(4)]
                    for ct in range(CT):
                        for j in range(4):
                            nc.tensor.matmul(
                                psVs[j][:, :],
                                xt_sb[:, ct, (rtc * 4 + j) * 128:(rtc * 4 + j + 1) * 128],
                                wv_sb[:, ct, :],
                                start=(ct == 0), stop=(ct == CT - 1),
                            )
                    for j in range(4):
                        nc.vector.tensor_copy(
                            v_sb[:, rtc * 4 + j, :, 0:HD],
                            psVs[j][:, :].rearrange("p (h e) -> p h e", h=HEADS_PER_CORE),
                        )

            # ---- phases B/C/D per q-chunk ----
            with (
                tc.tile_pool(name="ofpool", bufs=2) as ofpool,
                tc.tile_pool(name="psS", bufs=2, space="PSUM") as psS_pool,
                tc.tile_pool(name="psO", bufs=3, space="PSUM") as psO_pool,
                tc.tile_pool(name="psP", bufs=1, space="PSUM") as psP_pool,
            ):
                for qc in range(QC):
                    qsl = slice(qc * QCS, (qc + 1) * QCS)
                    on_sb = npool.tile([128, 2, QCS], F16, tag="on")
                    for pr in range(2):
                        psO_e = psO_pool.tile([128, QCS], F32, tag="psO")
                        psO_o = psO_pool.tile([128, QCS], F32, tag="psO")
                        for kt in range(NT):
                            ksl = slice(kt * 128, (kt + 1) * 128)
                            psS = psS_pool.tile([128, 2 * QCS], F32, tag="psS")
                            nc.tensor.matmul(
                                psS[:, 0:QCS],
                                qk_sb[0:64, 2 + pr, ksl],
                                qk_sb[0:64, pr, qsl],
                                start=True, stop=True,
                            )
                            nc.tensor.matmul(
                                psS[:, QCS:2 * QCS],
                                qk_sb[64:128, 2 + pr, ksl],
                                qk_sb[64:128, pr, qsl],
                                start=True, stop=True,
                            )
                            expt = work.tile([128, 2 * QCS], F16, tag="expt")
                            nc.scalar.activation(
                                expt[:, :], psS[:, :],
                                mybir.ActivationFunctionType.Exp,
                                bias=0.0, scale=SCALE,
                            )
                            nc.tensor.matmul(
                                psO_e[:, :],
                                v_sb[:, kt, 2 * pr, 0:128],
                                expt[:, 0:QCS],
                                start=(kt == 0), stop=(kt == NT - 1),
                            )
                            nc.tensor.matmul(
                                psO_o[:, :],
                                v_sb[:, kt, 2 * pr + 1, 0:128],
                                expt[:, QCS:2 * QCS],
                                start=(kt == 0), stop=(kt == NT - 1),
                            )
                        # normalize: o / rowsum  (rowsum is psO[64], per q position)
                        for hh, psO in ((0, psO_e), (1, psO_o)):
                            recip = npool.tile([65, QCS], F32, tag="recip")
                            nc.vector.reciprocal(recip[64:65, :], psO[64:65, :])
                            row_dram = dram.tile([1, QCS], F32, tag="row")
                            nc.sync.dma_start(out=row_dram[:, :], in_=recip[64:65, :])
                            rd = row_dram[:, :]
                            bcast_src = bass.AP(
                                tensor=rd.tensor, offset=rd.offset,
                                ap=[[0, 64]] + list(rd.ap[1:]),
                            )
                            bcast = npool.tile([64, QCS], F32, tag="bcast")
                            nc.sync.dma_start(out=bcast[:, :], in_=bcast_src)
                            nc.vector.tensor_mul(
                                on_sb[hh * 64:(hh + 1) * 64, pr, :],
                                psO[0:64, :],
                                bcast[:, :],
                            )
                    # ---- all-gather attention outputs (512KB, mesh regime), then
                    # compute this core's output-column slice of the projection ----
                    og = dram.tile([256, QCS], F16, tag="og")
                    nc.sync.dma_start(out=og[0:128, :], in_=on_sb[:, 0, :])
                    nc.sync.dma_start(out=og[128:256, :], in_=on_sb[:, 1, :])
                    ag = dram.tile([C, QCS], F16, tag="ag")
                    nc.gpsimd.collective_compute(
                        "AllGather",
                        mybir.AluOpType.bypass,
                        replica_groups=GROUPS,
                        ins=[og.opt()],
                        outs=[ag.opt()],
                    )
                    of_sb = ofpool.tile([128, CT, QCS], F16, tag="of")
                    ag_r = ag[:, :].rearrange("(t p) n -> t p n", p=128)
                    for t in range(CT):
                        nc.sync.dma_start(out=of_sb[:, t, :], in_=ag_r[t])
                    for m2 in range(2):
                        psP = psP_pool.tile([128, QCS], F32, tag="psP")
                        for kt8 in range(CT):
                            nc.tensor.matmul(
                                psP[:, :],
                                wp_sb[:, kt8, m2 * 128:(m2 + 1) * 128],
                                of_sb[:, kt8, :],
                                start=(kt8 == 0), stop=(kt8 == CT - 1),
                            )
                        outsb = work.tile([128, QCS], F16, tag="outsb")
                        nc.vector.tensor_scalar_add(outsb[:, :], psP[:, :], bc_sb[:, m2:m2 + 1])
                        nc.sync.dma_start(out=out_ext[m2 * 128:(m2 + 1) * 128, qsl], in_=outsb[:, :])

    nc.compile()
    return nc


def _get_nc():
    global _NC_CACHE
    if _NC_CACHE is None:
        _NC_CACHE = build()
    return _NC_CACHE


def shard_inputs(x, w_qkv, w_proj, b_proj):
    x = np.asarray(x, dtype=np.float32)
    w_qkv = np.asarray(w_qkv, dtype=np.float32)
    w_proj = np.asarray(w_proj, dtype=np.float32)
    b_proj = np.asarray(b_proj, dtype=np.float32)
    in_maps = []
    for core in range(8):
        b, g = divmod(core, 4)
        cs = slice(g * 256, (g + 1) * 256)
        wqk = np.concatenate([w_qkv[:, 0 * C + g * 256:0 * C + (g + 1) * 256],
                              w_qkv[:, 1 * C + g * 256:1 * C + (g + 1) * 256]], axis=1)
        in_maps.append({
            "xt": np.ascontiguousarray(x[b].T.astype(np.float16)),
            "wqk": np.ascontiguousarray(wqk.astype(np.float16)),
            "wv": np.ascontiguousarray(w_qkv[:, 2 * C + g * 256:2 * C + (g + 1) * 256].astype(np.float16)),
            "wpc": np.ascontiguousarray(w_proj[:, cs].astype(np.float16)),
            "bc": np.ascontiguousarray(b_proj[cs].reshape(2, 128).T),
            "ones64": np.ones((128, 64), dtype=np.float16),
            "zeros63": np.zeros((128, 63), dtype=np.float16),
        })
    return in_maps


def assemble_output(results):
    outT = np.empty((B, C, N), dtype=np.float32)
    for core in range(8):
        b, g = divmod(core, 4)
        outT[b, g * 256:(g + 1) * 256, :] = np.asarray(results[core]["out"], dtype=np.float32)
    return np.ascontiguousarray(outT.transpose(0, 2, 1))


def run_sharded(x, w_qkv, w_proj, b_proj, trace=False):
    nc = _get_nc()
    in_maps = shard_inputs(x, w_qkv, w_proj, b_proj)
    res = run_bass_kernel_spmd(nc, in_maps, core_ids=list(range(8)), trace=trace)
    return assemble_output(res.results), res.exec_time_ns


def kernel(x, w_qkv, w_proj, b_proj):
    out, _ = run_sharded(x, w_qkv, w_proj, b_proj, trace=False)
    return out


# revision 22
# speedup vs baseline: 1.6416x; 1.0154x over previous
"""Multi-head attention block (B=2, N=2048, C=1024, H=16, hd=64) on 8 TRN2 NeuronCores.

Sharding: data-parallel over batch (2 groups of 4 cores), tensor-parallel over
heads within each group (4 heads/core). Each core computes q/k/v for its heads,
attention, and a partial output projection; a ReduceScatter over the 4-core
group sums the partials, and the host reassembles the full [2, 2048, 1024]
output from the per-core shards.

Per-core layouts (everything transposed so the contraction dim sits on SBUF
partitions; the host pre-transposes x):
  xt   [1024, 2048]  x[b].T
  wqk  [1024, 512]   w_qkv columns for this core's q (256) ++ k (256)
  wv   [1024, 256]   w_qkv columns for this core's v
  wpb  [256, 1024]   w_proj rows for this core's heads
  bq   [128, 8]      b_proj/4, bq[p, m] = b_proj[m*128+p]/4
  out  [256, 2048]   rows g*256:(g+1)*256 of (x[b] @ ... ).T after RS
"""
import sys

if '/opt/trn_rl_repo' not in sys.path:
    sys.path.insert(0, '/opt/trn_rl_repo')

import numpy as np

import concourse.bass as bass
import concourse.mybir as mybir
import concourse.tile as tile
from concourse import bacc
from concourse.bass_utils import run_bass_kernel_spmd

F32 = mybir.dt.float32
F32R = mybir.dt.float32r
BF16 = mybir.dt.bfloat16
F16 = mybir.dt.float16

B = 2
N = 2048          # sequence length
C = 1024          # model dim
HEADS_PER_CORE = 4
HD = 64           # head dim
SCALE = HD ** -0.5
NT = N // 128     # 16 row tiles
CT = C // 128     # 8 contraction tiles
QC = 4            # q-chunks of 512
QCS = N // QC     # 512
GROUPS = [[0, 1, 2, 3], [4, 5, 6, 7]]

_NC_CACHE = None


def build():
    nc = bacc.Bacc(None, target_bir_lowering=False, debug=False)

    xt_ext = nc.declare_dram_parameter("xt", [C, N], F16, isOutput=False)
    wqk_ext = nc.declare_dram_parameter("wqk", [C, 512], F16, isOutput=False)
    wv_ext = nc.declare_dram_parameter("wv", [C, 256], F16, isOutput=False)
    wpc_ext = nc.declare_dram_parameter("wpc", [C, 256], F16, isOutput=False)
    bc_ext = nc.declare_dram_parameter("bc", [128, 2], F32, isOutput=False)
    ones_ext = nc.declare_dram_parameter("ones64", [128, 64], F16, isOutput=False)
    zeros_ext = nc.declare_dram_parameter("zeros63", [128, 63], F16, isOutput=False)
    out_ext = nc.declare_dram_parameter("out", [256, N], F16, isOutput=True)

    with tile.TileContext(nc) as tc:
        with (
            tc.tile_pool(name="weights", bufs=1) as wpool,
            tc.tile_pool(name="acts", bufs=1) as apool,
            tc.tile_pool(name="work", bufs=4) as work,
            tc.tile_pool(name="norm", bufs=2) as npool,
            tc.tile_pool(name="dram", bufs=2, space="DRAM") as dram,
        ):
            # ---- load inputs ----
            wqk_sb = wpool.tile([128, CT, 512], F16, tag="wqk")
            wv_sb = wpool.tile([128, CT, 256], F16, tag="wv")
            wp_sb = wpool.tile([128, CT, 256], F16, tag="wp")
            bc_sb = wpool.tile([128, 2], F32, tag="bc")

            wqk_r = wqk_ext.ap().rearrange("(t p) n -> t p n", p=128)
            wv_r = wv_ext.ap().rearrange("(t p) n -> t p n", p=128)
            wpc_r = wpc_ext.ap().rearrange("(t p) n -> t p n", p=128)


            # ---- phase A: qkT = wqk.T @ xt   [512, 2048], v = xt.T @ wv [2048, 256+ones] ----
            qk_sb = apool.tile([128, 4, N], F16, tag="qk")
            v_sb = apool.tile([128, NT, HEADS_PER_CORE, 128], F16, tag="v")
            # ones column for the row-sum trick (memset can't write f32r tiles;
            # DMA from a host constant instead)
            nc.sync.dma_start(
                out=v_sb[:, :, :, HD:HD + 1],
                in_=ones_ext.ap().rearrange("p (a b c) -> p a b c", a=NT, b=HEADS_PER_CORE),
            )
            # cols HD+1..127 of each v block are never-read pad (FWL needs a
            # 128-col stationary); zero them so no stale NaNs enter PSUM
            nc.sync.dma_start(
                out=v_sb[:, :, :, HD + 1:128],
                in_=bass.AP(tensor=zeros_ext.ap().tensor, offset=0,
                            ap=[[63, 128], [0, NT * HEADS_PER_CORE], [1, 63]]),
            )
            with (
                tc.tile_pool(name="xtpool", bufs=1) as xtpool,
                tc.tile_pool(name="psA", bufs=4, space="PSUM") as psA_pool,
            ):
                xt_sb = xtpool.tile([128, CT, N], F16, tag="xt")
                xt_r = xt_ext.ap().rearrange("(t p) n -> t p n", p=128)
                for ct in range(CT):
                    nc.sync.dma_start(out=xt_sb[:, ct, :], in_=xt_r[ct])
                    nc.sync.dma_start(out=wqk_sb[:, ct, :], in_=wqk_r[ct])
                    nc.sync.dma_start(out=wv_sb[:, ct, :], in_=wv_r[ct])
                    nc.sync.dma_start(out=wp_sb[:, ct, :], in_=wpc_r[ct])
                nc.sync.dma_start(out=bc_sb[:, :], in_=bc_ext[:, :])
                for m in range(4):
                    psAs = [psA_pool.tile([128, QCS], F32, tag="psA", name=f"psA_{m}_{i}") for i in range(QC)]
                    for ct in range(CT):
                        for qn in range(QC):
                            nc.tensor.matmul(
                                psAs[qn][:, :],
                                wqk_sb[:, ct, m * 128:(m + 1) * 128],
                                xt_sb[:, ct, qn * QCS:(qn + 1) * QCS],
                                start=(ct == 0), stop=(ct == CT - 1),
                            )
                    for qn in range(QC):
                        nc.vector.tensor_copy(qk_sb[:, m, qn * QCS:(qn + 1) * QCS], psAs[qn][:, :])
                for rtc in range(NT // 4):
                    psVs = [psA_pool.tile([128, 256], F32, tag="psA", name=f"psV_{rtc}_{i}") for i in range(4)]
                    for ct in range(CT):
                        for j in range(4):
                            nc.tensor.matmul(
                                psVs[j][:, :],
                                xt_sb[:, ct, (rtc * 4 + j) * 128:(rtc * 4 + j + 1) * 128],
                                wv_sb[:, ct, :],
                                start=(ct == 0), stop=(ct == CT - 1),
                            )
                    for j in range(4):
                        nc.vector.tensor_copy(
                            v_sb[:, rtc * 4 + j, :, 0:HD],
                            psVs[j][:, :].rearrange("p (h e) -> p h e", h=HEADS_PER_CORE),
                        )

            # ---- phases B/C/D per q-chunk ----
            with (
                tc.tile_pool(name="ofpool", bufs=2) as ofpool,
                tc.tile_pool(name="psS", bufs=2, space="PSUM") as psS_pool,
                tc.tile_pool(name="psO", bufs=3, space="PSUM") as psO_pool,
                tc.tile_pool(name="psP", bufs=1, space="PSUM") as psP_pool,
            ):
                for qc in range(QC):
                    qsl = slice(qc * QCS, (qc + 1) * QCS)
                    on_sb = npool.tile([128, 2, QCS], F16, tag="on")
                    def scores(pr, kt):
                        ksl = slice(kt * 128, (kt + 1) * 128)
                        psS = psS_pool.tile([128, 2 * QCS], F32, tag="psS",
                                            name=f"psS_{qc}_{pr}_{kt}")
                        nc.tensor.matmul(
                            psS[:, 0:QCS],
                            qk_sb[0:64, 2 + pr, ksl],
                            qk_sb[0:64, pr, qsl],
                            start=True, stop=True,
                        )
                        nc.tensor.matmul(
                            psS[:, QCS:2 * QCS],
                            qk_sb[64:128, 2 + pr, ksl],
                            qk_sb[64:128, pr, qsl],
                            start=True, stop=True,
                        )
                        return psS

                    for pr in range(2):
                        psO_e = psO_pool.tile([128, QCS], F32, tag="psO")
                        psO_o = psO_pool.tile([128, QCS], F32, tag="psO")
                        # 1-deep software pipeline: emit scores(kt+1) before the
                        # AV matmuls of kt so the in-order PE never sits behind
                        # the exp of kt (ACT is the pacing engine in this loop)
                        if pr == 0:
                            psS_cur = scores(pr, 0)
                        for kt in range(NT):
                            psS_next = scores(pr, kt + 1) if kt + 1 < NT else (
                                scores(pr + 1, 0) if pr == 0 else None)
                            expt = work.tile([128, 2 * QCS], F16, tag="expt")
                            nc.scalar.activation(
                                expt[:, :], psS_cur[:, :],
                                mybir.ActivationFunctionType.Exp,
                                bias=0.0, scale=SCALE,
                            )
                            nc.tensor.matmul(
                                psO_e[:, :],
                                v_sb[:, kt, 2 * pr, 0:128],
                                expt[:, 0:QCS],
                                start=(kt == 0), stop=(kt == NT - 1),
                            )
                            nc.tensor.matmul(
                                psO_o[:, :],
                                v_sb[:, kt, 2 * pr + 1, 0:128],
                                expt[:, QCS:2 * QCS],
                                start=(kt == 0), stop=(kt == NT - 1),
                            )
                            psS_cur = psS_next
                        # normalize: o / rowsum  (rowsum is psO[64], per q position)
                        for hh, psO in ((0, psO_e), (1, psO_o)):
                            recip = npool.tile([65, QCS], F32, tag="recip")
                            nc.vector.reciprocal(recip[64:65, :], psO[64:65, :])
                            row_dram = dram.tile([1, QCS], F32, tag="row")
                            nc.sync.dma_start(out=row_dram[:, :], in_=recip[64:65, :])
                            rd = row_dram[:, :]
                            bcast_src = bass.AP(
                                tensor=rd.tensor, offset=rd.offset,
                                ap=[[0, 64]] + list(rd.ap[1:]),
                            )
                            bcast = npool.tile([64, QCS], F32, tag="bcast")
                            nc.sync.dma_start(out=bcast[:, :], in_=bcast_src)
                            nc.vector.tensor_mul(
                                on_sb[hh * 64:(hh + 1) * 64, pr, :],
                                psO[0:64, :],
                                bcast[:, :],
                            )
                    # ---- all-gather attention outputs (512KB, mesh regime), then
                    # compute this core's output-column slice of the projection ----
                    og = dram.tile([256, QCS], F16, tag="og")
                    nc.sync.dma_start(out=og[0:128, :], in_=on_sb[:, 0, :])
                    nc.sync.dma_start(out=og[128:256, :], in_=on_sb[:, 1, :])
                    ag = dram.tile([C, QCS], F16, tag="ag")
                    nc.gpsimd.collective_compute(
                        "AllGather",
                        mybir.AluOpType.bypass,
                        replica_groups=GROUPS,
                        ins=[og.opt()],
                        outs=[ag.opt()],
                    )
                    of_sb = ofpool.tile([128, CT, QCS], F16, tag="of")
                    ag_r = ag[:, :].rearrange("(t p) n -> t p n", p=128)
                    for t in range(CT):
                        nc.sync.dma_start(out=of_sb[:, t, :], in_=ag_r[t])
                    for m2 in range(2):
                        psP = psP_pool.tile([128, QCS], F32, tag="psP")
                        for kt8 in range(CT):
                            nc.tensor.matmul(
                                psP[:, :],
                                wp_sb[:, kt8, m2 * 128:(m2 + 1) * 128],
                                of_sb[:, kt8, :],
                                start=(kt8 == 0), stop=(kt8 == CT - 1),
                            )
                        outsb = work.tile([128, QCS], F16, tag="outsb")
                        nc.vector.tensor_scalar_add(outsb[:, :], psP[:, :], bc_sb[:, m2:m2 + 1])
                        nc.sync.dma_start(out=out_ext[m2 * 128:(m2 + 1) * 128, qsl], in_=outsb[:, :])

    nc.compile()
    return nc


def _get_nc():
    global _NC_CACHE
    if _NC_CACHE is None:
        _NC_CACHE = build()
    return _NC_CACHE


def shard_inputs(x, w_qkv, w_proj, b_proj):
    x = np.asarray(x, dtype=np.float32)
    w_qkv = np.asarray(w_qkv, dtype=np.float32)
    w_proj = np.asarray(w_proj, dtype=np.float32)
    b_proj = np.asarray(b_proj, dtype=np.float32)
    in_maps = []
    for core in range(8):
        b, g = divmod(core, 4)
        cs = slice(g * 256, (g + 1) * 256)
        wqk = np.concatenate([w_qkv[:, 0 * C + g * 256:0 * C + (g + 1) * 256],
                              w_qkv[:, 1 * C + g * 256:1 * C + (g + 1) * 256]], axis=1)
        in_maps.append({
            "xt": np.ascontiguousarray(x[b].T.astype(np.float16)),
            "wqk": np.ascontiguousarray(wqk.astype(np.float16)),
            "wv": np.ascontiguousarray(w_qkv[:, 2 * C + g * 256:2 * C + (g + 1) * 256].astype(np.float16)),
            "wpc": np.ascontiguousarray(w_proj[:, cs].astype(np.float16)),
            "bc": np.ascontiguousarray(b_proj[cs].reshape(2, 128).T),
            "ones64": np.ones((128, 64), dtype=np.float16),
            "zeros63": np.zeros((128, 63), dtype=np.float16),
        })
    return in_maps


def assemble_output(results):
    outT = np.empty((B, C, N), dtype=np.float32)
    for core in range(8):
        b, g = divmod(core, 4)
        outT[b, g * 256:(g + 1) * 256, :] = np.asarray(results[core]["out"], dtype=np.float32)
    return np.ascontiguousarray(outT.transpose(0, 2, 1))


def run_sharded(x, w_qkv, w_proj, b_proj, trace=False):
    nc = _get_nc()
    in_maps = shard_inputs(x, w_qkv, w_proj, b_proj)
    res = run_bass_kernel_spmd(nc, in_maps, core_ids=list(range(8)), trace=trace)
    return assemble_output(res.results), res.exec_time_ns


def kernel(x, w_qkv, w_proj, b_proj):
    out, _ = run_sharded(x, w_qkv, w_proj, b_proj, trace=False)
    return out


# revision 23
# speedup vs baseline: 1.6590x; 1.0107x over previous
"""Multi-head attention block (B=2, N=2048, C=1024, H=16, hd=64) on 8 TRN2 NeuronCores.

Sharding: data-parallel over batch (2 groups of 4 cores), tensor-parallel over
heads within each group (4 heads/core). Each core computes q/k/v for its heads,
attention, and a partial output projection; a ReduceScatter over the 4-core
group sums the partials, and the host reassembles the full [2, 2048, 1024]
output from the per-core shards.

Per-core layouts (everything transposed so the contraction dim sits on SBUF
partitions; the host pre-transposes x):
  xt   [1024, 2048]  x[b].T
  wqk  [1024, 512]   w_qkv columns for this core's q (256) ++ k (256)
  wv   [1024, 256]   w_qkv columns for this core's v
  wpb  [256, 1024]   w_proj rows for this core's heads
  bq   [128, 8]      b_proj/4, bq[p, m] = b_proj[m*128+p]/4
  out  [256, 2048]   rows g*256:(g+1)*256 of (x[b] @ ... ).T after RS
"""
import sys

if '/opt/trn_rl_repo' not in sys.path:
    sys.path.insert(0, '/opt/trn_rl_repo')

import numpy as np

import concourse.bass as bass
import concourse.mybir as mybir
import concourse.tile as tile
from concourse import bacc
from concourse.bass_utils import run_bass_kernel_spmd

F32 = mybir.dt.float32
F32R = mybir.dt.float32r
BF16 = mybir.dt.bfloat16
F16 = mybir.dt.float16

B = 2
N = 2048          # sequence length
C = 1024          # model dim
HEADS_PER_CORE = 4
HD = 64           # head dim
SCALE = HD ** -0.5
NT = N // 128     # 16 row tiles
CT = C // 128     # 8 contraction tiles
QC = 4            # q-chunks of 512
QCS = N // QC     # 512
GROUPS = [[0, 1, 2, 3], [4, 5, 6, 7]]

_NC_CACHE = None


def build():
    nc = bacc.Bacc(None, target_bir_lowering=False, debug=False)

    xt_ext = nc.declare_dram_parameter("xt", [C, N], F16, isOutput=False)
    wqk_ext = nc.declare_dram_parameter("wqk", [C, 512], F16, isOutput=False)
    wv_ext = nc.declare_dram_parameter("wv", [C, 256], F16, isOutput=False)
    wpc_ext = nc.declare_dram_parameter("wpc", [C, 256], F16, isOutput=False)
    bc_ext = nc.declare_dram_parameter("bc", [128, 2], F32, isOutput=False)
    ones_ext = nc.declare_dram_parameter("ones64", [128, 64], F16, isOutput=False)
    zeros_ext = nc.declare_dram_parameter("zeros63", [128, 63], F16, isOutput=False)
    out_ext = nc.declare_dram_parameter("out", [256, N], F16, isOutput=True)

    with tile.TileContext(nc) as tc:
        with (
            tc.tile_pool(name="weights", bufs=1) as wpool,
            tc.tile_pool(name="acts", bufs=1) as apool,
            tc.tile_pool(name="work", bufs=4) as work,
            tc.tile_pool(name="norm", bufs=2) as npool,
            tc.tile_pool(name="dram", bufs=2, space="DRAM") as dram,
        ):
            # ---- load inputs ----
            wqk_sb = wpool.tile([128, CT, 512], F16, tag="wqk")
            wv_sb = wpool.tile([128, CT, 256], F16, tag="wv")
            wp_sb = wpool.tile([128, CT, 256], F16, tag="wp")
            bc_sb = wpool.tile([128, 2], F32, tag="bc")

            wqk_r = wqk_ext.ap().rearrange("(t p) n -> t p n", p=128)
            wv_r = wv_ext.ap().rearrange("(t p) n -> t p n", p=128)
            wpc_r = wpc_ext.ap().rearrange("(t p) n -> t p n", p=128)


            # ---- phase A: qkT = wqk.T @ xt   [512, 2048], v = xt.T @ wv [2048, 256+ones] ----
            qk_sb = apool.tile([128, 4, N], F16, tag="qk")
            v_sb = apool.tile([128, NT, HEADS_PER_CORE, 128], F16, tag="v")
            # ones column for the row-sum trick (memset can't write f32r tiles;
            # DMA from a host constant instead)
            nc.sync.dma_start(
                out=v_sb[:, :, :, HD:HD + 1],
                in_=ones_ext.ap().rearrange("p (a b c) -> p a b c", a=NT, b=HEADS_PER_CORE),
            )
            # cols HD+1..127 of each v block are never-read pad (FWL needs a
            # 128-col stationary); zero them so no stale NaNs enter PSUM
            nc.sync.dma_start(
                out=v_sb[:, :, :, HD + 1:128],
                in_=bass.AP(tensor=zeros_ext.ap().tensor, offset=0,
                            ap=[[63, 128], [0, NT * HEADS_PER_CORE], [1, 63]]),
            )
            with (
                tc.tile_pool(name="xtpool", bufs=1) as xtpool,
                tc.tile_pool(name="psA", bufs=4, space="PSUM") as psA_pool,
            ):
                xt_sb = xtpool.tile([128, CT, N], F16, tag="xt")
                xt_r = xt_ext.ap().rearrange("(t p) n -> t p n", p=128)
                for ct in range(CT):
                    nc.sync.dma_start(out=xt_sb[:, ct, :], in_=xt_r[ct])
                    nc.sync.dma_start(out=wqk_sb[:, ct, :], in_=wqk_r[ct])
                    nc.sync.dma_start(out=wv_sb[:, ct, :], in_=wv_r[ct])
                    nc.sync.dma_start(out=wp_sb[:, ct, :], in_=wpc_r[ct])
                nc.sync.dma_start(out=bc_sb[:, :], in_=bc_ext[:, :])
                for m in range(4):
                    psAs = [psA_pool.tile([128, QCS], F32, tag="psA", name=f"psA_{m}_{i}") for i in range(QC)]
                    for ct in range(CT):
                        for qn in range(QC):
                            nc.tensor.matmul(
                                psAs[qn][:, :],
                                wqk_sb[:, ct, m * 128:(m + 1) * 128],
                                xt_sb[:, ct, qn * QCS:(qn + 1) * QCS],
                                start=(ct == 0), stop=(ct == CT - 1),
                            )
                    for qn in range(QC):
                        nc.vector.tensor_copy(qk_sb[:, m, qn * QCS:(qn + 1) * QCS], psAs[qn][:, :])
                for rtc in range(NT // 4):
                    psVs = [psA_pool.tile([128, 256], F32, tag="psA", name=f"psV_{rtc}_{i}") for i in range(4)]
                    for ct in range(CT):
                        for j in range(4):
                            nc.tensor.matmul(
                                psVs[j][:, :],
                                xt_sb[:, ct, (rtc * 4 + j) * 128:(rtc * 4 + j + 1) * 128],
                                wv_sb[:, ct, :],
                                start=(ct == 0), stop=(ct == CT - 1),
                            )
                    for j in range(4):
                        nc.vector.tensor_copy(
                            v_sb[:, rtc * 4 + j, :, 0:HD],
                            psVs[j][:, :].rearrange("p (h e) -> p h e", h=HEADS_PER_CORE),
                        )

            # ---- phases B/C/D per q-chunk ----
            with (
                tc.tile_pool(name="ofpool", bufs=2) as ofpool,
                tc.tile_pool(name="psS", bufs=2, space="PSUM") as psS_pool,
                tc.tile_pool(name="psO", bufs=3, space="PSUM") as psO_pool,
                tc.tile_pool(name="psP", bufs=1, space="PSUM") as psP_pool,
            ):
                for qc in range(QC):
                    qsl = slice(qc * QCS, (qc + 1) * QCS)
                    ags = []
                    def scores(pr, kt):
                        ksl = slice(kt * 128, (kt + 1) * 128)
                        psS = psS_pool.tile([128, 2 * QCS], F32, tag="psS",
                                            name=f"psS_{qc}_{pr}_{kt}")
                        nc.tensor.matmul(
                            psS[:, 0:QCS],
                            qk_sb[0:64, 2 + pr, ksl],
                            qk_sb[0:64, pr, qsl],
                            start=True, stop=True,
                        )
                        nc.tensor.matmul(
                            psS[:, QCS:2 * QCS],
                            qk_sb[64:128, 2 + pr, ksl],
                            qk_sb[64:128, pr, qsl],
                            start=True, stop=True,
                        )
                        return psS

                    for pr in range(2):
                        on_sb = npool.tile([128, QCS], F16, tag="on")
                        psO_e = psO_pool.tile([128, QCS], F32, tag="psO")
                        psO_o = psO_pool.tile([128, QCS], F32, tag="psO")
                        # 1-deep software pipeline: emit scores(kt+1) before the
                        # AV matmuls of kt so the in-order PE never sits behind
                        # the exp of kt (ACT is the pacing engine in this loop)
                        if pr == 0:
                            psS_cur = scores(pr, 0)
                        for kt in range(NT):
                            psS_next = scores(pr, kt + 1) if kt + 1 < NT else (
                                scores(pr + 1, 0) if pr == 0 else None)
                            expt = work.tile([128, 2 * QCS], F16, tag="expt")
                            nc.scalar.activation(
                                expt[:, :], psS_cur[:, :],
                                mybir.ActivationFunctionType.Exp,
                                bias=0.0, scale=SCALE,
                            )
                            nc.tensor.matmul(
                                psO_e[:, :],
                                v_sb[:, kt, 2 * pr, 0:128],
                                expt[:, 0:QCS],
                                start=(kt == 0), stop=(kt == NT - 1),
                            )
                            nc.tensor.matmul(
                                psO_o[:, :],
                                v_sb[:, kt, 2 * pr + 1, 0:128],
                                expt[:, QCS:2 * QCS],
                                start=(kt == 0), stop=(kt == NT - 1),
                            )
                            psS_cur = psS_next
                        # normalize: o / rowsum  (rowsum is psO[64], per q position).
                        # Copy PSUM->SBUF first so the PSUM slot frees before the
                        # slow [1,512] reciprocal (keeps the in-order PE stream fed).
                        for hh, psO in ((0, psO_e), (1, psO_o)):
                            o_sb = npool.tile([65, QCS], F32, tag="o_sb")
                            nc.vector.tensor_copy(o_sb[:, :], psO[0:65, :])
                            recip = npool.tile([65, QCS], F32, tag="recip")
                            nc.vector.reciprocal(recip[64:65, :], o_sb[64:65, :])
                            row_dram = dram.tile([1, QCS], F32, tag="row")
                            nc.sync.dma_start(out=row_dram[:, :], in_=recip[64:65, :])
                            rd = row_dram[:, :]
                            bcast_src = bass.AP(
                                tensor=rd.tensor, offset=rd.offset,
                                ap=[[0, 64]] + list(rd.ap[1:]),
                            )
                            bcast = npool.tile([64, QCS], F32, tag="bcast")
                            nc.sync.dma_start(out=bcast[:, :], in_=bcast_src)
                            nc.vector.tensor_mul(
                                on_sb[hh * 64:(hh + 1) * 64, :],
                                o_sb[0:64, :],
                                bcast[:, :],
                            )
                        # all-gather this pair's attention outputs (128KB, mesh
                        # regime) while the other pair computes
                        og = dram.tile([128, QCS], F16, tag="og", name=f"og_{qc}_{pr}")
                        nc.sync.dma_start(out=og[:, :], in_=on_sb[:, :])
                        ag = dram.tile([512, QCS], F16, tag="ag", name=f"ag_{qc}_{pr}")
                        nc.gpsimd.collective_compute(
                            "AllGather",
                            mybir.AluOpType.bypass,
                            replica_groups=GROUPS,
                            ins=[og.opt()],
                            outs=[ag.opt()],
                        )
                        ags.append(ag)
                    # ---- projection on gathered heads (wpc rows are host-permuted
                    # to match the [pr, rank, head] gather order) ----
                    of_sbs = []
                    for pr in range(2):
                        of_sb = ofpool.tile([128, 4, QCS], F16, tag="of", name=f"of_{qc}_{pr}")
                        ag_r = ags[pr][:, :].rearrange("(t p) n -> t p n", p=128)
                        for t in range(4):
                            nc.sync.dma_start(out=of_sb[:, t, :], in_=ag_r[t])
                        of_sbs.append(of_sb)
                    for m2 in range(2):
                        psP = psP_pool.tile([128, QCS], F32, tag="psP")
                        for kt8 in range(CT):
                            nc.tensor.matmul(
                                psP[:, :],
                                wp_sb[:, kt8, m2 * 128:(m2 + 1) * 128],
                                of_sbs[kt8 // 4][:, kt8 % 4, :],
                                start=(kt8 == 0), stop=(kt8 == CT - 1),
                            )
                        outsb = work.tile([128, QCS], F16, tag="outsb")
                        nc.vector.tensor_scalar_add(outsb[:, :], psP[:, :], bc_sb[:, m2:m2 + 1])
                        nc.sync.dma_start(out=out_ext[m2 * 128:(m2 + 1) * 128, qsl], in_=outsb[:, :])

    nc.compile()
    return nc


def _get_nc():
    global _NC_CACHE
    if _NC_CACHE is None:
        _NC_CACHE = build()
    return _NC_CACHE


def shard_inputs(x, w_qkv, w_proj, b_proj):
    x = np.asarray(x, dtype=np.float32)
    w_qkv = np.asarray(w_qkv, dtype=np.float32)
    w_proj = np.asarray(w_proj, dtype=np.float32)
    b_proj = np.asarray(b_proj, dtype=np.float32)
    # ag row order: for each pr, rank-major then local-head-major:
    # rows [j*128 + h2*64 + e] <-> global head 4j + 2*pr + h2
    perm = np.concatenate([
        np.arange(1024).reshape(16, 64)[[4 * j + 2 * pr + h2 for j in range(4) for h2 in range(2)]].reshape(-1)
        for pr in range(2)
    ])
    in_maps = []
    for core in range(8):
        b, g = divmod(core, 4)
        cs = slice(g * 256, (g + 1) * 256)
        wqk = np.concatenate([w_qkv[:, 0 * C + g * 256:0 * C + (g + 1) * 256],
                              w_qkv[:, 1 * C + g * 256:1 * C + (g + 1) * 256]], axis=1)
        in_maps.append({
            "xt": np.ascontiguousarray(x[b].T.astype(np.float16)),
            "wqk": np.ascontiguousarray(wqk.astype(np.float16)),
            "wv": np.ascontiguousarray(w_qkv[:, 2 * C + g * 256:2 * C + (g + 1) * 256].astype(np.float16)),
            "wpc": np.ascontiguousarray(w_proj[perm, :][:, cs].astype(np.float16)),
            "bc": np.ascontiguousarray(b_proj[cs].reshape(2, 128).T),
            "ones64": np.ones((128, 64), dtype=np.float16),
            "zeros63": np.zeros((128, 63), dtype=np.float16),
        })
    return in_maps


def assemble_output(results):
    outT = np.empty((B, C, N), dtype=np.float32)
    for core in range(8):
        b, g = divmod(core, 4)
        outT[b, g * 256:(g + 1) * 256, :] = np.asarray(results[core]["out"], dtype=np.float32)
    return np.ascontiguousarray(outT.transpose(0, 2, 1))


def run_sharded(x, w_qkv, w_proj, b_proj, trace=False):
    nc = _get_nc()
    in_maps = shard_inputs(x, w_qkv, w_proj, b_proj)
    res = run_bass_kernel_spmd(nc, in_maps, core_ids=list(range(8)), trace=trace)
    return assemble_output(res.results), res.exec_time_ns


def kernel(x, w_qkv, w_proj, b_proj):
    out, _ = run_sharded(x, w_qkv, w_proj, b_proj, trace=False)
    return out


# revision 25
# speedup vs baseline: 1.8729x; 1.1289x over previous
"""Multi-head attention block (B=2, N=2048, C=1024, H=16, hd=64) on 8 TRN2 NeuronCores.

Sharding: data-parallel over batch (2 groups of 4 cores), tensor-parallel over
heads within each group (4 heads/core). Each core computes q/k/v for its heads,
attention, and a partial output projection; a ReduceScatter over the 4-core
group sums the partials, and the host reassembles the full [2, 2048, 1024]
output from the per-core shards.

Per-core layouts (everything transposed so the contraction dim sits on SBUF
partitions; the host pre-transposes x):
  xt   [1024, 2048]  x[b].T
  wqk  [1024, 512]   w_qkv columns for this core's q (256) ++ k (256)
  wv   [1024, 256]   w_qkv columns for this core's v
  wpb  [256, 1024]   w_proj rows for this core's heads
  bq   [128, 8]      b_proj/4, bq[p, m] = b_proj[m*128+p]/4
  out  [256, 2048]   rows g*256:(g+1)*256 of (x[b] @ ... ).T after RS
"""
import sys

if '/opt/trn_rl_repo' not in sys.path:
    sys.path.insert(0, '/opt/trn_rl_repo')

import numpy as np

import concourse.bass as bass
import concourse.mybir as mybir
import concourse.tile as tile
from concourse import bacc
from concourse.bass_utils import run_bass_kernel_spmd

F32 = mybir.dt.float32
F32R = mybir.dt.float32r
BF16 = mybir.dt.bfloat16
F16 = mybir.dt.float16

B = 2
N = 2048          # sequence length
C = 1024          # model dim
HEADS_PER_CORE = 4
HD = 64           # head dim
SCALE = HD ** -0.5
NT = N // 128     # 16 row tiles
CT = C // 128     # 8 contraction tiles
QC = 4            # q-chunks of 512
QCS = N // QC     # 512
GROUPS = [[0, 1, 2, 3], [4, 5, 6, 7]]

_NC_CACHE = None


def build():
    nc = bacc.Bacc(None, target_bir_lowering=False, debug=False)

    xt_ext = nc.declare_dram_parameter("xt", [C, N], F16, isOutput=False)
    wqk_ext = nc.declare_dram_parameter("wqk", [C, 512], F16, isOutput=False)
    wv_ext = nc.declare_dram_parameter("wv", [C, 256], F16, isOutput=False)
    wpc_ext = nc.declare_dram_parameter("wpc", [C, 256], F16, isOutput=False)
    bc_ext = nc.declare_dram_parameter("bc", [128, 2], F32, isOutput=False)
    ones_ext = nc.declare_dram_parameter("ones64", [128, 64], F16, isOutput=False)
    zeros_ext = nc.declare_dram_parameter("zeros63", [128, 63], F16, isOutput=False)
    out_ext = nc.declare_dram_parameter("out", [256, N], F16, isOutput=True)

    with tile.TileContext(nc) as tc:
        with (
            tc.tile_pool(name="weights", bufs=1) as wpool,
            tc.tile_pool(name="acts", bufs=1) as apool,
            tc.tile_pool(name="work", bufs=4) as work,
            tc.tile_pool(name="norm", bufs=2) as npool,
            tc.tile_pool(name="dram", bufs=2, space="DRAM") as dram,
            tc.tile_pool(name="dram4", bufs=4, space="DRAM") as dram4,
        ):
            # ---- load inputs ----
            wqk_sb = wpool.tile([128, CT, 512], F16, tag="wqk")
            wv_sb = wpool.tile([128, CT, 256], F16, tag="wv")
            wp_sb = wpool.tile([128, CT, 256], F16, tag="wp")
            bc_sb = wpool.tile([128, 2], F32, tag="bc")

            wqk_r = wqk_ext.ap().rearrange("(t p) n -> t p n", p=128)
            wv_r = wv_ext.ap().rearrange("(t p) n -> t p n", p=128)
            wpc_r = wpc_ext.ap().rearrange("(t p) n -> t p n", p=128)


            # ---- phase A: qkT = wqk.T @ xt   [512, 2048], v = xt.T @ wv [2048, 256+ones] ----
            qk_sb = apool.tile([128, 4, N], F16, tag="qk")
            v_sb = apool.tile([128, NT, HEADS_PER_CORE, 128], F16, tag="v")
            # ones column for the row-sum trick (memset can't write f32r tiles;
            # DMA from a host constant instead)
            nc.sync.dma_start(
                out=v_sb[:, :, :, HD:HD + 1],
                in_=ones_ext.ap().rearrange("p (a b c) -> p a b c", a=NT, b=HEADS_PER_CORE),
            )
            # cols HD+1..127 of each v block are never-read pad (FWL needs a
            # 128-col stationary); zero them so no stale NaNs enter PSUM
            nc.sync.dma_start(
                out=v_sb[:, :, :, HD + 1:128],
                in_=bass.AP(tensor=zeros_ext.ap().tensor, offset=0,
                            ap=[[63, 128], [0, NT * HEADS_PER_CORE], [1, 63]]),
            )
            with (
                tc.tile_pool(name="xtpool", bufs=1) as xtpool,
                tc.tile_pool(name="psA", bufs=4, space="PSUM") as psA_pool,
            ):
                xt_sb = xtpool.tile([128, CT, N], F16, tag="xt")
                xt_r = xt_ext.ap().rearrange("(t p) n -> t p n", p=128)
                for ct in range(CT):
                    nc.sync.dma_start(out=xt_sb[:, ct, :], in_=xt_r[ct])
                    nc.sync.dma_start(out=wqk_sb[:, ct, :], in_=wqk_r[ct])
                    nc.sync.dma_start(out=wv_sb[:, ct, :], in_=wv_r[ct])
                    nc.sync.dma_start(out=wp_sb[:, ct, :], in_=wpc_r[ct])
                nc.sync.dma_start(out=bc_sb[:, :], in_=bc_ext[:, :])
                for m in range(4):
                    psAs = [psA_pool.tile([128, QCS], F32, tag="psA", name=f"psA_{m}_{i}") for i in range(QC)]
                    for ct in range(CT):
                        for qn in range(QC):
                            nc.tensor.matmul(
                                psAs[qn][:, :],
                                wqk_sb[:, ct, m * 128:(m + 1) * 128],
                                xt_sb[:, ct, qn * QCS:(qn + 1) * QCS],
                                start=(ct == 0), stop=(ct == CT - 1),
                            )
                    for qn in range(QC):
                        nc.vector.tensor_copy(qk_sb[:, m, qn * QCS:(qn + 1) * QCS], psAs[qn][:, :])
                for rtc in range(NT // 4):
                    psVs = [psA_pool.tile([128, 256], F32, tag="psA", name=f"psV_{rtc}_{i}") for i in range(4)]
                    for ct in range(CT):
                        for j in range(4):
                            nc.tensor.matmul(
                                psVs[j][:, :],
                                xt_sb[:, ct, (rtc * 4 + j) * 128:(rtc * 4 + j + 1) * 128],
                                wv_sb[:, ct, :],
                                start=(ct == 0), stop=(ct == CT - 1),
                            )
                    for j in range(4):
                        nc.vector.tensor_copy(
                            v_sb[:, rtc * 4 + j, :, 0:HD],
                            psVs[j][:, :].rearrange("p (h e) -> p h e", h=HEADS_PER_CORE),
                        )

            # ---- phases B/C/D per q-chunk ----
            with (
                tc.tile_pool(name="ofpool", bufs=2) as ofpool,
                tc.tile_pool(name="psS", bufs=2, space="PSUM") as psS_pool,
                tc.tile_pool(name="psO", bufs=3, space="PSUM") as psO_pool,
                tc.tile_pool(name="psP", bufs=1, space="PSUM") as psP_pool,
            ):
                def scores(qc, pr, kt):
                    qsl_ = slice(qc * QCS, (qc + 1) * QCS)
                    ksl = slice(kt * 128, (kt + 1) * 128)
                    psS = psS_pool.tile([128, 2 * QCS], F32, tag="psS",
                                        name=f"psS_{qc}_{pr}_{kt}")
                    nc.tensor.matmul(
                        psS[:, 0:QCS],
                        qk_sb[0:64, 2 + pr, ksl],
                        qk_sb[0:64, pr, qsl_],
                        start=True, stop=True,
                    )
                    nc.tensor.matmul(
                        psS[:, QCS:2 * QCS],
                        qk_sb[64:128, 2 + pr, ksl],
                        qk_sb[64:128, pr, qsl_],
                        start=True, stop=True,
                    )
                    return psS

                ags = {}

                def do_proj(qc):
                    qsl_ = slice(qc * QCS, (qc + 1) * QCS)
                    of_sbs = []
                    for pr in range(2):
                        of_sb = ofpool.tile([128, 4, QCS], F16, tag="of", name=f"of_{qc}_{pr}")
                        ag_r = ags[(qc, pr)][:, :].rearrange("(t p) n -> t p n", p=128)
                        for t in range(4):
                            nc.sync.dma_start(out=of_sb[:, t, :], in_=ag_r[t])
                        of_sbs.append(of_sb)
                    for m2 in range(2):
                        psP = psP_pool.tile([128, QCS], F32, tag="psP", name=f"psP_{qc}_{m2}")
                        for kt8 in range(CT):
                            nc.tensor.matmul(
                                psP[:, :],
                                wp_sb[:, kt8, m2 * 128:(m2 + 1) * 128],
                                of_sbs[kt8 // 4][:, kt8 % 4, :],
                                start=(kt8 == 0), stop=(kt8 == CT - 1),
                            )
                        outsb = work.tile([128, QCS], F16, tag="outsb", name=f"outsb_{qc}_{m2}")
                        nc.vector.tensor_scalar_add(outsb[:, :], psP[:, :], bc_sb[:, m2:m2 + 1])
                        nc.sync.dma_start(out=out_ext[m2 * 128:(m2 + 1) * 128, qsl_], in_=outsb[:, :])

                blocks = [(qc, pr) for qc in range(QC) for pr in range(2)]
                psS_cur = scores(0, 0, 0)
                for bi, (qc, pr) in enumerate(blocks):
                    on_sb = npool.tile([128, QCS], F16, tag="on", name=f"on_{qc}_{pr}")
                    psO_e = psO_pool.tile([128, QCS], F32, tag="psO", name=f"psOe_{qc}_{pr}")
                    psO_o = psO_pool.tile([128, QCS], F32, tag="psO", name=f"psOo_{qc}_{pr}")
                    for kt in range(NT):
                        # 1-deep software pipeline across block boundaries: the
                        # in-order PE must never sit directly behind exp(kt)
                        if kt + 1 < NT:
                            nxt = (qc, pr, kt + 1)
                        elif bi + 1 < len(blocks):
                            nxt = (blocks[bi + 1][0], blocks[bi + 1][1], 0)
                        else:
                            nxt = None
                        psS_next = scores(*nxt) if nxt else None
                        expt = work.tile([128, 2 * QCS], F16, tag="expt",
                                         name=f"expt_{qc}_{pr}_{kt}")
                        nc.scalar.activation(
                            expt[:, :], psS_cur[:, :],
                            mybir.ActivationFunctionType.Exp,
                            bias=0.0, scale=SCALE,
                        )
                        nc.tensor.matmul(
                            psO_e[:, :],
                            v_sb[:, kt, 2 * pr, 0:128],
                            expt[:, 0:QCS],
                            start=(kt == 0), stop=(kt == NT - 1),
                        )
                        nc.tensor.matmul(
                            psO_o[:, :],
                            v_sb[:, kt, 2 * pr + 1, 0:128],
                            expt[:, QCS:2 * QCS],
                            start=(kt == 0), stop=(kt == NT - 1),
                        )
                        psS_cur = psS_next
                    # normalize: o / rowsum (rowsum is psO[64], per q position).
                    # Copy PSUM->SBUF first so the PSUM slot frees before the
                    # slow [1,512] reciprocal.
                    for hh, psO in ((0, psO_e), (1, psO_o)):
                        o_sb = npool.tile([65, QCS], F32, tag="o_sb", name=f"osb_{qc}_{pr}_{hh}")
                        nc.vector.tensor_copy(o_sb[:, :], psO[0:65, :])
                        recip = npool.tile([65, QCS], F32, tag="recip", name=f"rc_{qc}_{pr}_{hh}")
                        nc.vector.reciprocal(recip[64:65, :], o_sb[64:65, :])
                        row_dram = dram.tile([1, QCS], F32, tag="row", name=f"row_{qc}_{pr}_{hh}")
                        nc.sync.dma_start(out=row_dram[:, :], in_=recip[64:65, :])
                        rd = row_dram[:, :]
                        bcast_src = bass.AP(
                            tensor=rd.tensor, offset=rd.offset,
                            ap=[[0, 64]] + list(rd.ap[1:]),
                        )
                        bcast = npool.tile([64, QCS], F32, tag="bcast", name=f"bc_{qc}_{pr}_{hh}")
                        nc.sync.dma_start(out=bcast[:, :], in_=bcast_src)
                        nc.vector.tensor_mul(
                            on_sb[hh * 64:(hh + 1) * 64, :],
                            o_sb[0:64, :],
                            bcast[:, :],
                        )
                    # all-gather this pair's attention outputs (128KB, mesh regime)
                    og = dram.tile([128, QCS], F16, tag="og", name=f"og_{qc}_{pr}")
                    nc.sync.dma_start(out=og[:, :], in_=on_sb[:, :])
                    ag = dram4.tile([512, QCS], F16, tag="ag", name=f"ag_{qc}_{pr}")
                    nc.gpsimd.collective_compute(
                        "AllGather",
                        mybir.AluOpType.bypass,
                        replica_groups=GROUPS,
                        ins=[og.opt()],
                        outs=[ag.opt()],
                    )
                    ags[(qc, pr)] = ag
                    # deferred projection: qc-1's AG completed during this qc's
                    # attention, so its matmuls never stall the in-order PE
                    if pr == 1 and qc > 0:
                        do_proj(qc - 1)
                do_proj(QC - 1)

    nc.compile()
    return nc


def _get_nc():
    global _NC_CACHE
    if _NC_CACHE is None:
        _NC_CACHE = build()
    return _NC_CACHE


def shard_inputs(x, w_qkv, w_proj, b_proj):
    x = np.asarray(x, dtype=np.float32)
    w_qkv = np.asarray(w_qkv, dtype=np.float32)
    w_proj = np.asarray(w_proj, dtype=np.float32)
    b_proj = np.asarray(b_proj, dtype=np.float32)
    # ag row order: for each pr, rank-major then local-head-major:
    # rows [j*128 + h2*64 + e] <-> global head 4j + 2*pr + h2
    perm = np.concatenate([
        np.arange(1024).reshape(16, 64)[[4 * j + 2 * pr + h2 for j in range(4) for h2 in range(2)]].reshape(-1)
        for pr in range(2)
    ])
    in_maps = []
    for core in range(8):
        b, g = divmod(core, 4)
        cs = slice(g * 256, (g + 1) * 256)
        wqk = np.concatenate([w_qkv[:, 0 * C + g * 256:0 * C + (g + 1) * 256],
                              w_qkv[:, 1 * C + g * 256:1 * C + (g + 1) * 256]], axis=1)
        in_maps.append({
            "xt": np.ascontiguousarray(x[b].T.astype(np.float16)),
            "wqk": np.ascontiguousarray(wqk.astype(np.float16)),
            "wv": np.ascontiguousarray(w_qkv[:, 2 * C + g * 256:2 * C + (g + 1) * 256].astype(np.float16)),
            "wpc": np.ascontiguousarray(w_proj[perm, :][:, cs].astype(np.float16)),
            "bc": np.ascontiguousarray(b_proj[cs].reshape(2, 128).T),
            "ones64": np.ones((128, 64), dtype=np.float16),
            "zeros63": np.zeros((128, 63), dtype=np.float16),
        })
    return in_maps


def assemble_output(results):
    outT = np.empty((B, C, N), dtype=np.float32)
    for core in range(8):
        b, g = divmod(core, 4)
        outT[b, g * 256:(g + 1) * 256, :] = np.asarray(results[core]["out"], dtype=np.float32)
    return np.ascontiguousarray(outT.transpose(0, 2, 1))


def run_sharded(x, w_qkv, w_proj, b_proj, trace=False):
    nc = _get_nc()
    in_maps = shard_inputs(x, w_qkv, w_proj, b_proj)
    res = run_bass_kernel_spmd(nc, in_maps, core_ids=list(range(8)), trace=trace)
    return assemble_output(res.results), res.exec_time_ns


def kernel(x, w_qkv, w_proj, b_proj):
    out, _ = run_sharded(x, w_qkv, w_proj, b_proj, trace=False)
    return out


# revision 26
# speedup vs baseline: 1.8860x; 1.0070x over previous
"""Multi-head attention block (B=2, N=2048, C=1024, H=16, hd=64) on 8 TRN2 NeuronCores.

Sharding: data-parallel over batch (2 groups of 4 cores), tensor-parallel over
heads within each group (4 heads/core). Each core computes q/k/v for its heads,
attention, and a partial output projection; a ReduceScatter over the 4-core
group sums the partials, and the host reassembles the full [2, 2048, 1024]
output from the per-core shards.

Per-core layouts (everything transposed so the contraction dim sits on SBUF
partitions; the host pre-transposes x):
  xt   [1024, 2048]  x[b].T
  wqk  [1024, 512]   w_qkv columns for this core's q (256) ++ k (256)
  wv   [1024, 256]   w_qkv columns for this core's v
  wpb  [256, 1024]   w_proj rows for this core's heads
  bq   [128, 8]      b_proj/4, bq[p, m] = b_proj[m*128+p]/4
  out  [256, 2048]   rows g*256:(g+1)*256 of (x[b] @ ... ).T after RS
"""
import sys

if '/opt/trn_rl_repo' not in sys.path:
    sys.path.insert(0, '/opt/trn_rl_repo')

import numpy as np

import concourse.bass as bass
import concourse.mybir as mybir
import concourse.tile as tile
from concourse import bacc
from concourse.bass_utils import run_bass_kernel_spmd

F32 = mybir.dt.float32
F32R = mybir.dt.float32r
BF16 = mybir.dt.bfloat16
F16 = mybir.dt.float16

B = 2
N = 2048          # sequence length
C = 1024          # model dim
HEADS_PER_CORE = 4
HD = 64           # head dim
SCALE = HD ** -0.5
NT = N // 128     # 16 row tiles
CT = C // 128     # 8 contraction tiles
QC = 4            # q-chunks of 512
QCS = N // QC     # 512
GROUPS = [[0, 1, 2, 3], [4, 5, 6, 7]]

_NC_CACHE = None


def build():
    nc = bacc.Bacc(None, target_bir_lowering=False, debug=False)

    xt_ext = nc.declare_dram_parameter("xt", [C, N], F16, isOutput=False)
    wqk_ext = nc.declare_dram_parameter("wqk", [C, 512], F16, isOutput=False)
    wv_ext = nc.declare_dram_parameter("wv", [C, 256], F16, isOutput=False)
    wpc_ext = nc.declare_dram_parameter("wpc", [C, 256], F16, isOutput=False)
    bc_ext = nc.declare_dram_parameter("bc", [128, 2], F32, isOutput=False)
    ones_ext = nc.declare_dram_parameter("ones64", [128, 64], F16, isOutput=False)
    zeros_ext = nc.declare_dram_parameter("zeros63", [128, 63], F16, isOutput=False)
    out_ext = nc.declare_dram_parameter("out", [256, N], F16, isOutput=True)

    with tile.TileContext(nc) as tc:
        with (
            tc.tile_pool(name="weights", bufs=1) as wpool,
            tc.tile_pool(name="acts", bufs=1) as apool,
            tc.tile_pool(name="work", bufs=4) as work,
            tc.tile_pool(name="norm", bufs=2) as npool,
            tc.tile_pool(name="dram", bufs=2, space="DRAM") as dram,
            tc.tile_pool(name="dram4", bufs=4, space="DRAM") as dram4,
        ):
            # ---- load inputs ----
            wqk_sb = wpool.tile([128, CT, 512], F16, tag="wqk")
            wv_sb = wpool.tile([128, CT, 256], F16, tag="wv")
            wp_sb = wpool.tile([128, CT, 256], F16, tag="wp")
            bc_sb = wpool.tile([128, 2], F32, tag="bc")

            wqk_r = wqk_ext.ap().rearrange("(t p) n -> t p n", p=128)
            wv_r = wv_ext.ap().rearrange("(t p) n -> t p n", p=128)
            wpc_r = wpc_ext.ap().rearrange("(t p) n -> t p n", p=128)


            # ---- phase A: qkT = wqk.T @ xt   [512, 2048], v = xt.T @ wv [2048, 256+ones] ----
            qk_sb = apool.tile([128, 4, N], F16, tag="qk")
            v_sb = apool.tile([128, NT, HEADS_PER_CORE, 128], F16, tag="v")
            # ones column for the row-sum trick (memset can't write f32r tiles;
            # DMA from a host constant instead)
            nc.sync.dma_start(
                out=v_sb[:, :, :, HD:HD + 1],
                in_=ones_ext.ap().rearrange("p (a b c) -> p a b c", a=NT, b=HEADS_PER_CORE),
            )
            # cols HD+1..127 of each v block are never-read pad (FWL needs a
            # 128-col stationary); zero them so no stale NaNs enter PSUM
            nc.sync.dma_start(
                out=v_sb[:, :, :, HD + 1:128],
                in_=bass.AP(tensor=zeros_ext.ap().tensor, offset=0,
                            ap=[[63, 128], [0, NT * HEADS_PER_CORE], [1, 63]]),
            )
            with (
                tc.tile_pool(name="psA", bufs=4, space="PSUM") as psA_pool,
            ):
                xt_sb = apool.tile([128, CT, N], F16, tag="xt")
                xt_r = xt_ext.ap().rearrange("(t p) n -> t p n", p=128)
                for ct in range(CT):
                    nc.sync.dma_start(out=xt_sb[:, ct, :], in_=xt_r[ct])
                    nc.sync.dma_start(out=wqk_sb[:, ct, :], in_=wqk_r[ct])
                    nc.sync.dma_start(out=wv_sb[:, ct, :], in_=wv_r[ct])
                    nc.sync.dma_start(out=wp_sb[:, ct, :], in_=wpc_r[ct])
                nc.sync.dma_start(out=bc_sb[:, :], in_=bc_ext[:, :])
                for m in (0, 2):
                    psAs = [psA_pool.tile([128, QCS], F32, tag="psA", name=f"psA_{m}_{i}") for i in range(QC)]
                    for ct in range(CT):
                        for qn in range(QC):
                            nc.tensor.matmul(
                                psAs[qn][:, :],
                                wqk_sb[:, ct, m * 128:(m + 1) * 128],
                                xt_sb[:, ct, qn * QCS:(qn + 1) * QCS],
                                start=(ct == 0), stop=(ct == CT - 1),
                            )
                    for qn in range(QC):
                        nc.vector.tensor_copy(qk_sb[:, m, qn * QCS:(qn + 1) * QCS], psAs[qn][:, :])
                for rtc in range(NT // 4):
                    psVs = [psA_pool.tile([128, 256], F32, tag="psA", name=f"psV_{rtc}_{i}") for i in range(4)]
                    for ct in range(CT):
                        for j in range(4):
                            nc.tensor.matmul(
                                psVs[j][:, :],
                                xt_sb[:, ct, (rtc * 4 + j) * 128:(rtc * 4 + j + 1) * 128],
                                wv_sb[:, ct, :],
                                start=(ct == 0), stop=(ct == CT - 1),
                            )
                    for j in range(4):
                        nc.vector.tensor_copy(
                            v_sb[:, rtc * 4 + j, :, 0:HD],
                            psVs[j][:, :].rearrange("p (h e) -> p h e", h=HEADS_PER_CORE),
                        )

            # ---- phases B/C/D per q-chunk ----
            with (
                tc.tile_pool(name="ofpool", bufs=4) as ofpool,
                tc.tile_pool(name="psS", bufs=2, space="PSUM") as psS_pool,
                tc.tile_pool(name="psO", bufs=3, space="PSUM") as psO_pool,
                tc.tile_pool(name="psP", bufs=1, space="PSUM") as psP_pool,
            ):
                def scores(qc, pr, kt):
                    qsl_ = slice(qc * QCS, (qc + 1) * QCS)
                    ksl = slice(kt * 128, (kt + 1) * 128)
                    psS = psS_pool.tile([128, 2 * QCS], F32, tag="psS",
                                        name=f"psS_{qc}_{pr}_{kt}")
                    nc.tensor.matmul(
                        psS[:, 0:QCS],
                        qk_sb[0:64, 2 + pr, ksl],
                        qk_sb[0:64, pr, qsl_],
                        start=True, stop=True,
                    )
                    nc.tensor.matmul(
                        psS[:, QCS:2 * QCS],
                        qk_sb[64:128, 2 + pr, ksl],
                        qk_sb[64:128, pr, qsl_],
                        start=True, stop=True,
                    )
                    return psS

                ags = {}

                def do_proj(qc):
                    qsl_ = slice(qc * QCS, (qc + 1) * QCS)
                    of_sbs = []
                    for pr in range(2):
                        of_sb = ofpool.tile([128, 4, QCS], F16, tag="of", name=f"of_{qc}_{pr}")
                        ag_r = ags[(qc, pr)][:, :].rearrange("(t p) n -> t p n", p=128)
                        for t in range(4):
                            nc.sync.dma_start(out=of_sb[:, t, :], in_=ag_r[t])
                        of_sbs.append(of_sb)
                    for m2 in range(2):
                        psP = psP_pool.tile([128, QCS], F32, tag="psP", name=f"psP_{qc}_{m2}")
                        for kt8 in range(CT):
                            nc.tensor.matmul(
                                psP[:, :],
                                wp_sb[:, kt8, m2 * 128:(m2 + 1) * 128],
                                of_sbs[kt8 // 4][:, kt8 % 4, :],
                                start=(kt8 == 0), stop=(kt8 == CT - 1),
                            )
                        outsb = work.tile([128, QCS], F16, tag="outsb", name=f"outsb_{qc}_{m2}")
                        nc.vector.tensor_scalar_add(outsb[:, :], psP[:, :], bc_sb[:, m2:m2 + 1])
                        nc.sync.dma_start(out=out_ext[m2 * 128:(m2 + 1) * 128, qsl_], in_=outsb[:, :])

                qk_pending = [(1, 0), (3, 0), (3, 1), (3, 2), (3, 3), (1, 1), (1, 2), (1, 3)]

                def emit_qk_group(m, qn):
                    psq = psP_pool.tile([128, QCS], F32, tag="psP", name=f"psq_{m}_{qn}")
                    for ct in range(CT):
                        nc.tensor.matmul(
                            psq[:, :],
                            wqk_sb[:, ct, m * 128:(m + 1) * 128],
                            xt_sb[:, ct, qn * QCS:(qn + 1) * QCS],
                            start=(ct == 0), stop=(ct == CT - 1),
                        )
                    nc.vector.tensor_copy(qk_sb[:, m, qn * QCS:(qn + 1) * QCS], psq[:, :])

                blocks = [(qc, pr) for qc in range(QC) for pr in range(2)]
                psS_cur = scores(0, 0, 0)
                for bi, (qc, pr) in enumerate(blocks):
                    on_sb = npool.tile([128, QCS], F16, tag="on", name=f"on_{qc}_{pr}")
                    psO_e = psO_pool.tile([128, QCS], F32, tag="psO", name=f"psOe_{qc}_{pr}")
                    psO_o = psO_pool.tile([128, QCS], F32, tag="psO", name=f"psOo_{qc}_{pr}")
                    for kt in range(NT):
                        # 1-deep software pipeline across block boundaries: the
                        # in-order PE must never sit directly behind exp(kt)
                        if kt + 1 < NT:
                            nxt = (qc, pr, kt + 1)
                        elif bi + 1 < len(blocks):
                            nxt = (blocks[bi + 1][0], blocks[bi + 1][1], 0)
                        else:
                            nxt = None
                        psS_next = scores(*nxt) if nxt else None
                        expt = work.tile([128, 2 * QCS], F16, tag="expt",
                                         name=f"expt_{qc}_{pr}_{kt}")
                        nc.scalar.activation(
                            expt[:, :], psS_cur[:, :],
                            mybir.ActivationFunctionType.Exp,
                            bias=0.0, scale=SCALE,
                        )
                        nc.tensor.matmul(
                            psO_e[:, :],
                            v_sb[:, kt, 2 * pr, 0:128],
                            expt[:, 0:QCS],
                            start=(kt == 0), stop=(kt == NT - 1),
                        )
                        nc.tensor.matmul(
                            psO_o[:, :],
                            v_sb[:, kt, 2 * pr + 1, 0:128],
                            expt[:, QCS:2 * QCS],
                            start=(kt == 0), stop=(kt == NT - 1),
                        )
                        psS_cur = psS_next
                        if bi == 0 and kt % 2 == 0 and qk_pending:
                            emit_qk_group(*qk_pending.pop(0))
                    # normalize: o / rowsum (rowsum is psO[64], per q position).
                    # Copy PSUM->SBUF first so the PSUM slot frees before the
                    # slow [1,512] reciprocal.
                    for hh, psO in ((0, psO_e), (1, psO_o)):
                        o_sb = npool.tile([65, QCS], F32, tag="o_sb", name=f"osb_{qc}_{pr}_{hh}")
                        nc.vector.tensor_copy(o_sb[:, :], psO[0:65, :])
                        recip = npool.tile([65, QCS], F32, tag="recip", name=f"rc_{qc}_{pr}_{hh}")
                        nc.vector.reciprocal(recip[64:65, :], o_sb[64:65, :])
                        row_dram = dram.tile([1, QCS], F32, tag="row", name=f"row_{qc}_{pr}_{hh}")
                        nc.sync.dma_start(out=row_dram[:, :], in_=recip[64:65, :])
                        rd = row_dram[:, :]
                        bcast_src = bass.AP(
                            tensor=rd.tensor, offset=rd.offset,
                            ap=[[0, 64]] + list(rd.ap[1:]),
                        )
                        bcast = npool.tile([64, QCS], F32, tag="bcast", name=f"bc_{qc}_{pr}_{hh}")
                        nc.sync.dma_start(out=bcast[:, :], in_=bcast_src)
                        nc.vector.tensor_mul(
                            on_sb[hh * 64:(hh + 1) * 64, :],
                            o_sb[0:64, :],
                            bcast[:, :],
                        )
                    # all-gather this pair's attention outputs (128KB, mesh regime)
                    og = dram.tile([128, QCS], F16, tag="og", name=f"og_{qc}_{pr}")
                    nc.sync.dma_start(out=og[:, :], in_=on_sb[:, :])
                    ag = dram4.tile([512, QCS], F16, tag="ag", name=f"ag_{qc}_{pr}")
                    nc.gpsimd.collective_compute(
                        "AllGather",
                        mybir.AluOpType.bypass,
                        replica_groups=GROUPS,
                        ins=[og.opt()],
                        outs=[ag.opt()],
                    )
                    ags[(qc, pr)] = ag
                    # deferred projection: qc-1's AG completed during this qc's
                    # attention, so its matmuls never stall the in-order PE
                    if pr == 1 and qc > 0:
                        do_proj(qc - 1)
                do_proj(QC - 1)

    nc.compile()
    return nc


def _get_nc():
    global _NC_CACHE
    if _NC_CACHE is None:
        _NC_CACHE = build()
    return _NC_CACHE


def shard_inputs(x, w_qkv, w_proj, b_proj):
    x = np.asarray(x, dtype=np.float32)
    w_qkv = np.asarray(w_qkv, dtype=np.float32)
    w_proj = np.asarray(w_proj, dtype=np.float32)
    b_proj = np.asarray(b_proj, dtype=np.float32)
    # ag row order: for each pr, rank-major then local-head-major:
    # rows [j*128 + h2*64 + e] <-> global head 4j + 2*pr + h2
    perm = np.concatenate([
        np.arange(1024).reshape(16, 64)[[4 * j + 2 * pr + h2 for j in range(4) for h2 in range(2)]].reshape(-1)
        for pr in range(2)
    ])
    in_maps = []
    for core in range(8):
        b, g = divmod(core, 4)
        cs = slice(g * 256, (g + 1) * 256)
        wqk = np.concatenate([w_qkv[:, 0 * C + g * 256:0 * C + (g + 1) * 256],
                              w_qkv[:, 1 * C + g * 256:1 * C + (g + 1) * 256]], axis=1)
        in_maps.append({
            "xt": np.ascontiguousarray(x[b].T.astype(np.float16)),
            "wqk": np.ascontiguousarray(wqk.astype(np.float16)),
            "wv": np.ascontiguousarray(w_qkv[:, 2 * C + g * 256:2 * C + (g + 1) * 256].astype(np.float16)),
            "wpc": np.ascontiguousarray(w_proj[perm, :][:, cs].astype(np.float16)),
            "bc": np.ascontiguousarray(b_proj[cs].reshape(2, 128).T),
            "ones64": np.ones((128, 64), dtype=np.float16),
            "zeros63": np.zeros((128, 63), dtype=np.float16),
        })
    return in_maps


def assemble_output(results):
    outT = np.empty((B, C, N), dtype=np.float32)
    for core in range(8):
        b, g = divmod(core, 4)
        outT[b, g * 256:(g + 1) * 256, :] = np.asarray(results[core]["out"], dtype=np.float32)
    return np.ascontiguousarray(outT.transpose(0, 2, 1))


def run_sharded(x, w_qkv, w_proj, b_proj, trace=False):
    nc = _get_nc()
    in_maps = shard_inputs(x, w_qkv, w_proj, b_proj)
    res = run_bass_kernel_spmd(nc, in_maps, core_ids=list(range(8)), trace=trace)
    return assemble_output(res.results), res.exec_time_ns


def kernel(x, w_qkv, w_proj, b_proj):
    out, _ = run_sharded(x, w_qkv, w_proj, b_proj, trace=False)
    return out
